# revision 21
# baseline (speedup 1.0000x reference)
"""Trainium2 Bass kernel for 2-layer GraphSAGE (mean aggregation) on 8 NeuronCores.

Strategy (graph/data parallel, dst-partitioned), v2:
  - Destination nodes sharded across 8 cores (12.5K each); edges partitioned by
    destination core, grouped by (128-wide dest tile, src-quarter), padded to
    128-edge chunks.
  - Whole feature path in bf16: gather tables, masks, weights, self features.
    PSUM accumulation stays f32.
  - Source features gathered from 4 global quarter-tables (rows = global node
    id % 25000, int16 indices) with SWDGE dma_gather, 4 queues.
  - Segment-mean as one-hot mask matmul on the TensorEngine: per 128-edge
    chunk, aggT[f, d] += g_chunk.T @ mask with mask[e, d] =
    (iota[d] == dst_local[e]) * invdeg[e] built by one DVE tensor_scalar.
  - Dense branch feature-major: hT = Wl.T-stationary @ aggT + Wr.T @ selfT;
    self features come pre-transposed from the host (x_selfT / h_selfT in
    DRAM), so no on-device transpose for the self term.
  - Layer 1 epilogue: one fused Silu activation (psum -> bf16 SBUF), store
    feature-major h_selfT for layer-2 self term, PE-transpose to row-major
    h_row for the collective.
  - Between layers: ONE AllGather of the full h shard (bf16, 25.6MB out)
    into the shared h_tab; layer-2 gathers depend on it.
"""

import numpy as np

# ---------------------------------------------------------------- problem dims
N_NODES = 100000
N_EDGES = 800000
D = 128
NC = 8

TILE = 128            # destination-tile width
NT = 100              # dest tiles per core (underfilled for group balance)
NSLOT = NT * TILE     # padded slots per core
QTBL = NSLOT * NC // 4  # rows per quarter gather table (fits int16 idx)
GATHER_BUFS = 8
NQ = 4                # SWDGE queues

_cache = {}


def _stile_sizes(nt, n_stiles=8):
    base = nt // n_stiles
    rem = nt - base * n_stiles
    return [base + (1 if i < rem else 0) for i in range(n_stiles)]


# ------------------------------------------------------------------- host plan
def _plan(edge_index, n_nodes, n_cores):
    src = np.asarray(edge_index[0], dtype=np.int64)
    dst = np.asarray(edge_index[1], dtype=np.int64)
    E = src.shape[0]

    nloc = n_nodes // n_cores
    nt = NT

    sizes = _stile_sizes(nt)
    stiles = []
    t0 = 0
    for s in sizes:
        stiles.append(list(range(t0, t0 + s)))
        t0 += s

    deg = np.bincount(dst, minlength=n_nodes).astype(np.float64)
    invdeg = (1.0 / np.maximum(deg, 1.0)).astype(np.float32)

    # Per-core slot permutation: greedily pack destinations (desc. degree)
    # into NT tiles minimizing the max per-quarter edge load, so
    # per-(tile, quarter) group sizes stay under 256 on every core.
    # slot_of[node] = owning-core-local slot in [0, NSLOT).
    slot_of = np.empty(n_nodes, dtype=np.int64)
    # per-dst per-quarter degree (quarter = src's global-slot quarter; but the
    # quarter of a source depends on slot assignment of ITS core, which this
    # loop is computing. Use the core-of-src (pairs of cores share a quarter
    # table) as the quarter key: quarter = gslot // QTBL = core(src) // 2.
    srcq = (src // nloc) // (n_cores // 4)
    d4 = np.zeros((n_nodes, 4), dtype=np.int64)
    np.add.at(d4, (dst, srcq), 1)
    for k in range(n_cores):
        d4k = d4[k * nloc:(k + 1) * nloc]
        order_k = np.argsort(-d4k.sum(1), kind="stable")
        loads = np.zeros((nt, 4), dtype=np.int64)
        cnt = np.zeros(nt, dtype=np.int64)
        tile_k = np.empty(nloc, dtype=np.int64)
        slot_in = np.empty(nloc, dtype=np.int64)
        for idx in order_k:
            cand = (loads + d4k[idx]).max(axis=1)
            cand[cnt >= TILE] = 1 << 30
            t = int(np.argmin(cand))
            tile_k[idx] = t
            slot_in[idx] = cnt[t]
            cnt[t] += 1
            loads[t] += d4k[idx]
        slot_of[k * nloc:(k + 1) * nloc] = tile_k * TILE + slot_in

    gslot = (np.arange(n_nodes) // nloc) * NSLOT + slot_of  # global slot row

    core = dst // nloc
    dslot = slot_of[dst]
    tile = dslot // TILE
    dtl = (dslot % TILE).astype(np.float32)
    sgs = gslot[src]
    quarter = sgs // QTBL
    tblrow = (sgs % QTBL).astype(np.int16)

    gid = (core * nt + tile) * 4 + quarter
    order = np.argsort(gid, kind="stable")
    counts = np.bincount(gid, minlength=n_cores * nt * 4).reshape(n_cores, nt, 4)
    gmax = counts.max(axis=0)                       # [nt, 4]
    gpad = ((gmax + 127) // 128) * 128

    goff = np.zeros((nt, 4), dtype=np.int64)
    pos = 0
    call_list = []                                  # (quarter, stile_idx, off, n)
    for si, tiles in enumerate(stiles):
        for q in range(4):
            call_off = pos
            for t in tiles:
                goff[t, q] = pos
                pos += int(gpad[t, q])
            call_list.append((q, si, call_off, pos - call_off))
    ep = pos

    idx_st = np.zeros((n_cores, ep), dtype=np.int16)
    dst_st = np.full((n_cores, ep), -1.0, dtype=np.float32)
    inv_st = np.zeros((n_cores, ep), dtype=np.float32)

    gid_s = gid[order]
    grp_start = np.searchsorted(gid_s, np.arange(n_cores * nt * 4))
    within = np.arange(E) - grp_start[gid_s]
    k_s = gid_s // (nt * 4)
    t_s = (gid_s // 4) % nt
    q_s = gid_s % 4
    put = goff[t_s, q_s] + within
    idx_st[k_s, put] = tblrow[order]
    dst_st[k_s, put] = dtl[order]
    inv_st[k_s, put] = invdeg[dst[order]]

    max_slots = max(n for (_, _, _, n) in call_list) // 128

    return dict(
        nloc=nloc, nt=nt, stiles=stiles, gpad=gpad,
        goff=goff, ep=ep, call_list=call_list, max_slots=max_slots,
        idx_st=idx_st, dst_st=dst_st, inv_st=inv_st, slot_of=slot_of,
        n_cores=n_cores, n_nodes=n_nodes,
    )


def _wrap16(stream):
    """[ep] -> [128, ep//16] wrapped-16 + replicated layout for dma_gather."""
    ep = stream.shape[0]
    w = stream.reshape(ep // 16, 16).T          # [16, ep//16]
    return np.tile(w, (8, 1))                   # [128, ep//16]


def _colmajor(stream):
    """[ep] -> [128, ep//128] with element j at [j%128, j//128]."""
    ep = stream.shape[0]
    return stream.reshape(ep // 128, 128).T.copy()


# --------------------------------------------------------------- bass builder
def _build(plan, iters=1):
    import concourse.bass as bass
    import concourse.tile as tile
    from concourse import bacc, mybir
    from concourse.library_config import mlp
    from concourse.tile_rust import add_dep_helper

    f32 = mybir.dt.float32
    bf16 = mybir.dt.bfloat16
    i16 = mybir.dt.int16

    nt = plan["nt"]; nloc_pad = NSLOT
    ep = plan["ep"]; gpad = plan["gpad"]; goff = plan["goff"]
    call_list = plan["call_list"]; stiles = plan["stiles"]
    n_cores = plan["n_cores"]
    max_slots = plan["max_slots"]

    nc = bacc.Bacc("TRN2", target_bir_lowering=False, debug=False,
                   num_swdge_queues=NQ)

    # inputs
    x_tabs = [nc.dram_tensor(f"x_tab{q}", [QTBL, D], bf16, kind="ExternalInput")
              for q in range(4)]
    x_selfT = nc.dram_tensor("x_selfT", [128, nloc_pad], bf16, kind="ExternalInput")
    idxs_in = nc.dram_tensor("idxs", [128, ep // 16], i16, kind="ExternalInput")
    dstv_in = nc.dram_tensor("dstv", [128, ep // 128], f32, kind="ExternalInput")
    invv_in = nc.dram_tensor("invv", [128, ep // 128], f32, kind="ExternalInput")
    iota_in = nc.dram_tensor("iota", [128, 128], bf16, kind="ExternalInput")
    ident_in = nc.dram_tensor("ident", [128, 128], bf16, kind="ExternalInput")
    w_in = {nm: nc.dram_tensor(nm, [128, 128], bf16, kind="ExternalInput")
            for nm in ("w1lt", "w1rt", "w2lt", "w2rt")}
    b_in = {nm: nc.dram_tensor(nm, [128, 1], f32, kind="ExternalInput")
            for nm in ("b1", "b2")}
    out_t = nc.dram_tensor("outT", [128, nloc_pad], f32, kind="ExternalOutput")

    # internal DRAM
    h_selfT = nc.dram_tensor("h_selfT", [128, nloc_pad], bf16)
    h_row = nc.dram_tensor("h_row", [nloc_pad, D], bf16)
    h_tab = nc.dram_tensor("h_tab", [NSLOT * n_cores, D], bf16,
                           addr_space="Shared")
    h_tabs = [h_tab[q * QTBL:(q + 1) * QTBL, :] for q in range(4)]

    silu = mybir.ActivationFunctionType.Silu
    copyf = mybir.ActivationFunctionType.Identity

    max_w = max(len(s) for s in stiles)

    with tile.TileContext(nc) as tc:
        lib_inst = nc.gpsimd.load_library(mlp)
        with (
            tc.tile_pool(name="persist", bufs=1) as pp,
            tc.tile_pool(name="gather", bufs=GATHER_BUFS) as gpo,
            tc.tile_pool(name="mask", bufs=16) as mpo,
            tc.tile_pool(name="agg", bufs=2) as apo,
            tc.tile_pool(name="selfp", bufs=2) as sfo,
            tc.tile_pool(name="hbuf", bufs=2) as hpo,
            tc.tile_pool(name="psA", bufs=4, space="PSUM") as psa,
            tc.tile_pool(name="psT", bufs=2, space="PSUM") as pst,
            tc.tile_pool(name="psH", bufs=2, space="PSUM") as psh,
        ):
            # persistent SBUF
            idx_sb = pp.tile([128, ep // 16], i16)
            nc.sync.dma_start(idx_sb[:], idxs_in[:])
            dstv_sb = pp.tile([128, ep // 128], f32)
            nc.sync.dma_start(dstv_sb[:], dstv_in[:])
            invv_sb = pp.tile([128, ep // 128], f32)
            nc.sync.dma_start(invv_sb[:], invv_in[:])
            iota_sb = pp.tile([128, 128], bf16)
            nc.sync.dma_start(iota_sb[:], iota_in[:])
            ident_sb = pp.tile([128, 128], bf16)
            nc.sync.dma_start(ident_sb[:], ident_in[:])
            w_sb = {}
            for nm, t in w_in.items():
                w_sb[nm] = pp.tile([128, 128], bf16, tag=nm, name=f"w_{nm}")
                nc.sync.dma_start(w_sb[nm][:], t[:])
            b_sb = {}
            for nm, t in b_in.items():
                b_sb[nm] = pp.tile([128, 1], f32, tag=nm, name=f"b_{nm}")
                nc.sync.dma_start(b_sb[nm][:], t[:])

            first_gather = [True]

            def layer(src_tabs, selfT_src, wl, wr, bias, is_last, ag_insts):
                """Emit one SAGE layer. Returns list of h-store instructions."""
                store_insts = []
                for si_idx, tiles in enumerate(stiles):
                    w = len(tiles)
                    t0 = tiles[0]
                    # issue the stile's 4 gather calls (parallel queues)
                    gbufs = {}
                    for (q, csi, off, n) in call_list:
                        if csi != si_idx:
                            continue
                        g = gpo.tile([128, max_slots, D], bf16, tag="g")
                        slots = n // 128
                        gi = nc.gpsimd.dma_gather(
                            g[:, :slots, :], src_tabs[q][:],
                            idx_sb[:, off // 16:(off + n) // 16],
                            n, n, D, queue_num=q,
                            single_packet=False)
                        if first_gather[0]:
                            add_dep_helper(gi.ins, lib_inst.ins, sync=True,
                                           reason="lib before gather")
                            first_gather[0] = False
                        if ag_insts is not None:
                            for ag in ag_insts:
                                add_dep_helper(gi.ins, ag.ins, sync=True,
                                               reason="gather after AG")
                        gbufs[q] = (g, off)
                    # whole-stile self features: one DMA
                    selfT = sfo.tile([128, max_w * 128], bf16, tag="selfT")
                    nc.sync.dma_start(
                        selfT[:, :w * 128],
                        selfT_src[:, t0 * 128:(t0 + w) * 128])
                    # aggregation per tile -> aggT_big slices
                    aggT = apo.tile([128, max_w * 128], bf16, tag="aggT")
                    for ti, t in enumerate(tiles):
                        chunk_cols = []
                        for q in range(4):
                            npads = int(gpad[t, q])
                            if npads == 0:
                                continue
                            g, off = gbufs[q]
                            base_slot = (int(goff[t, q]) - off) // 128
                            for ci in range(npads // 128):
                                col = int(goff[t, q]) // 128 + ci
                                chunk_cols.append((g, base_slot + ci, col))
                        ps = psa.tile([128, 128], f32, tag="psagg")
                        nchunks = len(chunk_cols)
                        for j, (g, slot, col) in enumerate(chunk_cols):
                            m = mpo.tile([128, 128], bf16, tag="m")
                            nc.vector.tensor_scalar(
                                m[:], iota_sb[:],
                                dstv_sb[:, col:col + 1],
                                invv_sb[:, col:col + 1],
                                mybir.AluOpType.is_equal,
                                mybir.AluOpType.mult)
                            nc.tensor.matmul(
                                ps[:], g[:, slot, :], m[:],
                                start=(j == 0), stop=(j == nchunks - 1))
                        nc.scalar.copy(aggT[:, ti * 128:(ti + 1) * 128], ps[:])
                    # dense + epilogue in 512-wide groups
                    hT = None
                    if not is_last:
                        hT = hpo.tile([128, max_w * 128], bf16, tag="hT")
                        hr = hpo.tile([128, max_w, D], bf16, tag="hr")
                    else:
                        o = hpo.tile([128, max_w * 128], f32, tag="o")
                    for c0 in range(0, w * 128, 512):
                        c1 = min(c0 + 512, w * 128)
                        ph = psh.tile([128, 512], f32, tag="psh")
                        nc.tensor.matmul(ph[:, :c1 - c0], wl[:],
                                         aggT[:, c0:c1],
                                         start=True, stop=False)
                        nc.tensor.matmul(ph[:, :c1 - c0], wr[:],
                                         selfT[:, c0:c1],
                                         start=False, stop=True)
                        if is_last:
                            nc.scalar.activation(o[:, c0:c1], ph[:, :c1 - c0],
                                                 copyf, bias=bias[:])
                        else:
                            nc.scalar.activation(hT[:, c0:c1], ph[:, :c1 - c0],
                                                 silu, bias=bias[:])
                    if is_last:
                        nc.sync.dma_start(
                            out_t[:, t0 * 128:(t0 + w) * 128], o[:, :w * 128])
                    else:
                        nc.sync.dma_start(
                            h_selfT[:, t0 * 128:(t0 + w) * 128], hT[:, :w * 128])
                        for ti in range(w):
                            pt = pst.tile([128, 128], bf16, tag="pst")
                            nc.tensor.transpose(
                                pt[:], hT[:, ti * 128:(ti + 1) * 128],
                                ident_sb[:])
                            nc.scalar.copy(hr[:, ti, :], pt[:])
                        sins = nc.sync.dma_start(
                            h_row[t0 * 128:(t0 + w) * 128, :].rearrange(
                                "(a p) f -> p a f", p=128),
                            hr[:, :w, :])
                        store_insts.append(sins)
                return store_insts

            for _ in range(iters):
                l1_stores = layer(x_tabs, x_selfT,
                                  w_sb["w1lt"], w_sb["w1rt"], b_sb["b1"],
                                  False, None)
                ag = nc.gpsimd.collective_compute(
                    "AllGather", mybir.AluOpType.bypass,
                    replica_groups=[list(range(n_cores))],
                    ins=[h_row[:, :]],
                    outs=[h_tab[:]])
                for sins in l1_stores:
                    add_dep_helper(ag.ins, sins.ins, sync=True,
                                   reason="AG after h stores")
                layer(h_tabs, h_selfT,
                      w_sb["w2lt"], w_sb["w2rt"], b_sb["b2"],
                      True, [ag])

    nc.compile()
    return nc


# ----------------------------------------------------------------- host entry
def _prepare(edge_index):
    return _plan(edge_index, N_NODES, NC)


def _in_maps(plan, x, w1l, w1r, b1, w2l, w2r, b2):
    import ml_dtypes
    bf16 = ml_dtypes.bfloat16

    x = np.asarray(x, dtype=np.float32)
    xb = x.astype(bf16)
    nloc = plan["nloc"]
    n_cores = plan["n_cores"]
    n_nodes = plan["n_nodes"]
    slot_of = plan["slot_of"]
    gslot = (np.arange(n_nodes) // nloc) * NSLOT + slot_of

    # slot-ordered global table (shared by all cores)
    tab_full = np.zeros((NSLOT * n_cores, D), dtype=bf16)
    tab_full[gslot] = xb
    tabs = [np.ascontiguousarray(tab_full[q * QTBL:(q + 1) * QTBL])
            for q in range(4)]
    iota = np.broadcast_to(np.arange(128, dtype=np.float32),
                           (128, 128)).astype(bf16)
    ident = np.eye(128, dtype=np.float32).astype(bf16)
    wts = {
        "w1lt": np.ascontiguousarray(np.asarray(w1l, np.float32).T.astype(bf16)),
        "w1rt": np.ascontiguousarray(np.asarray(w1r, np.float32).T.astype(bf16)),
        "w2lt": np.ascontiguousarray(np.asarray(w2l, np.float32).T.astype(bf16)),
        "w2rt": np.ascontiguousarray(np.asarray(w2r, np.float32).T.astype(bf16)),
    }
    maps = []
    for k in range(n_cores):
        xsT = np.zeros((128, NSLOT), dtype=bf16)
        xsT[:, slot_of[k * nloc:(k + 1) * nloc]] = xb[k * nloc:(k + 1) * nloc].T
        m = {
            "x_selfT": xsT,
            "idxs": _wrap16(plan["idx_st"][k]),
            "dstv": _colmajor(plan["dst_st"][k]),
            "invv": _colmajor(plan["inv_st"][k]),
            "iota": iota, "ident": ident,
            "b1": np.asarray(b1, np.float32).reshape(128, 1),
            "b2": np.asarray(b2, np.float32).reshape(128, 1),
        }
        m.update(wts)
        for q in range(4):
            m[f"x_tab{q}"] = tabs[q]
        maps.append(m)
    return maps


def _run(inputs, iters=1):
    """Compile (cached) and run; returns full [N, D] output."""
    from concourse.bass_utils import run_bass_kernel_spmd

    edge_index = np.asarray(inputs["edge_index"])
    key = ("k", iters, edge_index.shape[1])
    if key not in _cache:
        plan = _prepare(edge_index)
        nc = _build(plan, iters=iters)
        _cache[key] = (plan, nc)
    plan, nc = _cache[key]
    maps = _in_maps(plan, inputs["x"], inputs["W1_l"], inputs["W1_r"],
                    inputs["b1"], inputs["W2_l"], inputs["W2_r"], inputs["b2"])
    res = run_bass_kernel_spmd(nc, maps, core_ids=list(range(plan["n_cores"])))
    nloc = plan["nloc"]
    slot_of = plan["slot_of"]
    outs = [np.asarray(res.results[k]["outT"], dtype=np.float32).T[
                slot_of[k * nloc:(k + 1) * nloc]]
            for k in range(plan["n_cores"])]
    return np.concatenate(outs, axis=0)


def kernel(**inputs) -> np.ndarray:
    return _run(inputs, iters=1)


# revision 26
# speedup vs baseline: 1.6807x; 1.6807x over previous
"""Trainium2 Bass kernel for 2-layer GraphSAGE (mean aggregation) on 8 NeuronCores.

Strategy (graph/data parallel, dst-partitioned), v2:
  - Destination nodes sharded across 8 cores (12.5K each); edges partitioned by
    destination core, grouped by (128-wide dest tile, src-quarter), padded to
    128-edge chunks.
  - Whole feature path in bf16: gather tables, masks, weights, self features.
    PSUM accumulation stays f32.
  - Source features gathered from 4 global quarter-tables (rows = global node
    id % 25000, int16 indices) with SWDGE dma_gather, 4 queues.
  - Segment-mean as one-hot mask matmul on the TensorEngine: per 128-edge
    chunk, aggT[f, d] += g_chunk.T @ mask with mask[e, d] =
    (iota[d] == dst_local[e]) * invdeg[e] built by one DVE tensor_scalar.
  - Dense branch feature-major: hT = Wl.T-stationary @ aggT + Wr.T @ selfT;
    self features come pre-transposed from the host (x_selfT / h_selfT in
    DRAM), so no on-device transpose for the self term.
  - Layer 1 epilogue: one fused Silu activation (psum -> bf16 SBUF), store
    feature-major h_selfT for layer-2 self term, PE-transpose to row-major
    h_row for the collective.
  - Between layers: ONE AllGather of the full h shard (bf16, 25.6MB out)
    into the shared h_tab; layer-2 gathers depend on it.
"""

import numpy as np

# ---------------------------------------------------------------- problem dims
N_NODES = 100000
N_EDGES = 800000
D = 128
NC = 8

TILE = 128            # destination-tile width
NT = 100              # dest tiles per core (underfilled for group balance)
NSLOT = NT * TILE     # padded slots per core
QTBL = NSLOT * NC // 4  # rows per quarter gather table (fits int16 idx)
GATHER_BUFS = 8
NQ = 4                # SWDGE queues

_cache = {}


def _stile_sizes(nt, n_stiles=8):
    base = nt // n_stiles
    rem = nt - base * n_stiles
    return [base + (1 if i < rem else 0) for i in range(n_stiles)]


# ------------------------------------------------------------------- host plan
def _plan(edge_index, n_nodes, n_cores):
    src = np.asarray(edge_index[0], dtype=np.int64)
    dst = np.asarray(edge_index[1], dtype=np.int64)
    E = src.shape[0]

    nloc = n_nodes // n_cores
    nt = NT

    sizes = _stile_sizes(nt)
    stiles = []
    t0 = 0
    for s in sizes:
        stiles.append(list(range(t0, t0 + s)))
        t0 += s

    deg = np.bincount(dst, minlength=n_nodes).astype(np.float64)
    invdeg = (1.0 / np.maximum(deg, 1.0)).astype(np.float32)

    # Per-core slot permutation: greedily pack destinations (desc. degree)
    # into NT tiles minimizing the max per-quarter edge load, so
    # per-(tile, quarter) group sizes stay under 256 on every core.
    # slot_of[node] = owning-core-local slot in [0, NSLOT).
    slot_of = np.empty(n_nodes, dtype=np.int64)
    # per-dst per-quarter degree (quarter = src's global-slot quarter; but the
    # quarter of a source depends on slot assignment of ITS core, which this
    # loop is computing. Use the core-of-src (pairs of cores share a quarter
    # table) as the quarter key: quarter = gslot // QTBL = core(src) // 2.
    srcq = (src // nloc) // (n_cores // 4)
    d4 = np.zeros((n_nodes, 4), dtype=np.int64)
    np.add.at(d4, (dst, srcq), 1)
    for k in range(n_cores):
        d4k = d4[k * nloc:(k + 1) * nloc]
        order_k = np.argsort(-d4k.sum(1), kind="stable")
        loads = np.zeros((nt, 4), dtype=np.int64)
        cnt = np.zeros(nt, dtype=np.int64)
        tile_k = np.empty(nloc, dtype=np.int64)
        slot_in = np.empty(nloc, dtype=np.int64)
        for idx in order_k:
            cand = (loads + d4k[idx]).max(axis=1)
            cand[cnt >= TILE] = 1 << 30
            t = int(np.argmin(cand))
            tile_k[idx] = t
            slot_in[idx] = cnt[t]
            cnt[t] += 1
            loads[t] += d4k[idx]
        slot_of[k * nloc:(k + 1) * nloc] = tile_k * TILE + slot_in

    gslot = (np.arange(n_nodes) // nloc) * NSLOT + slot_of  # global slot row

    core = dst // nloc
    core_src = src // nloc
    dslot = slot_of[dst]
    tile = dslot // TILE
    dtl = (dslot % TILE).astype(np.float32)
    inve = invdeg[dst]

    def streams(quarter, tblrow):
        gid = (core * nt + tile) * 4 + quarter
        order = np.argsort(gid, kind="stable")
        counts = np.bincount(gid, minlength=n_cores * nt * 4).reshape(
            n_cores, nt, 4)
        gmax = counts.max(axis=0)                   # [nt, 4]
        gpad = ((gmax + 127) // 128) * 128

        goff = np.zeros((nt, 4), dtype=np.int64)
        pos = 0
        call_list = []                              # (quarter, stile_idx, off, n)
        for si, tiles in enumerate(stiles):
            for q in range(4):
                call_off = pos
                for t in tiles:
                    goff[t, q] = pos
                    pos += int(gpad[t, q])
                call_list.append((q, si, call_off, pos - call_off))
        ep = pos

        idx_st = np.zeros((n_cores, ep), dtype=np.int16)
        dst_st = np.full((n_cores, ep), -1.0, dtype=np.float32)
        inv_st = np.zeros((n_cores, ep), dtype=np.float32)

        gid_s = gid[order]
        grp_start = np.searchsorted(gid_s, np.arange(n_cores * nt * 4))
        within = np.arange(E) - grp_start[gid_s]
        k_s = gid_s // (nt * 4)
        t_s = (gid_s // 4) % nt
        q_s = gid_s % 4
        put = goff[t_s, q_s] + within
        idx_st[k_s, put] = tblrow[order]
        dst_st[k_s, put] = dtl[order]
        inv_st[k_s, put] = inve[order]
        max_slots = max(n for (_, _, _, n) in call_list) // 128
        return dict(gpad=gpad, goff=goff, ep=ep, call_list=call_list,
                    idx_st=idx_st, dst_st=dst_st, inv_st=inv_st,
                    max_slots=max_slots)

    # layer 1: absolute slot-table order
    sgs = gslot[src]
    s1 = streams(sgs // QTBL, (sgs % QTBL).astype(np.int16))
    # layer 2: receiver-relative core order (h_tab row block j holds the
    # shard of core (self ^ j))
    rel = (core_src ^ core) * NSLOT + slot_of[src]
    s2 = streams(rel // QTBL, (rel % QTBL).astype(np.int16))

    return dict(
        nloc=nloc, nt=nt, stiles=stiles, s1=s1, s2=s2, slot_of=slot_of,
        n_cores=n_cores, n_nodes=n_nodes,
    )


def _wrap16(stream):
    """[ep] -> [128, ep//16] wrapped-16 + replicated layout for dma_gather."""
    ep = stream.shape[0]
    w = stream.reshape(ep // 16, 16).T          # [16, ep//16]
    return np.tile(w, (8, 1))                   # [128, ep//16]


def _colmajor(stream):
    """[ep] -> [128, ep//128] with element j at [j%128, j//128]."""
    ep = stream.shape[0]
    return stream.reshape(ep // 128, 128).T.copy()


# --------------------------------------------------------------- bass builder
def _build(plan, iters=1):
    import os
    import concourse.bass as bass
    import concourse.tile as tile
    from concourse import bacc, mybir
    from concourse.library_config import mlp
    from concourse.tile_rust import add_dep_helper

    RDMA_AG = bool(int(os.environ.get("K_RDMA_AG", "1")))
    from concourse.library_config import remote_dma as remote_dma_lib

    f32 = mybir.dt.float32
    bf16 = mybir.dt.bfloat16
    i16 = mybir.dt.int16

    nt = plan["nt"]; nloc_pad = NSLOT
    stiles = plan["stiles"]
    n_cores = plan["n_cores"]
    s1, s2 = plan["s1"], plan["s2"]
    ep1, ep2 = s1["ep"], s2["ep"]

    nc = bacc.Bacc("TRN2", target_bir_lowering=False, debug=False,
                   num_swdge_queues=NQ + 1 if RDMA_AG else NQ)

    # inputs
    x_tabs = [nc.dram_tensor(f"x_tab{q}", [QTBL, D], bf16, kind="ExternalInput")
              for q in range(4)]
    x_selfT = nc.dram_tensor("x_selfT", [128, nloc_pad], bf16, kind="ExternalInput")
    idx1_in = nc.dram_tensor("idxs1", [128, ep1 // 16], i16, kind="ExternalInput")
    dstv1_in = nc.dram_tensor("dstv1", [128, ep1 // 128], f32, kind="ExternalInput")
    invv1_in = nc.dram_tensor("invv1", [128, ep1 // 128], f32, kind="ExternalInput")
    idx2_in = nc.dram_tensor("idxs2", [128, ep2 // 16], i16, kind="ExternalInput")
    dstv2_in = nc.dram_tensor("dstv2", [128, ep2 // 128], f32, kind="ExternalInput")
    invv2_in = nc.dram_tensor("invv2", [128, ep2 // 128], f32, kind="ExternalInput")
    iota_in = nc.dram_tensor("iota", [128, 128], bf16, kind="ExternalInput")
    ident_in = nc.dram_tensor("ident", [128, 128], bf16, kind="ExternalInput")
    w_in = {nm: nc.dram_tensor(nm, [128, 128], bf16, kind="ExternalInput")
            for nm in ("w1lt", "w1rt", "w2lt", "w2rt")}
    b_in = {nm: nc.dram_tensor(nm, [128, 1], f32, kind="ExternalInput")
            for nm in ("b1", "b2")}
    out_t = nc.dram_tensor("outT", [128, nloc_pad], f32, kind="ExternalOutput")

    # internal DRAM
    h_selfT = nc.dram_tensor("h_selfT", [128, nloc_pad], bf16)
    h_row = nc.dram_tensor("h_row", [nloc_pad, D], bf16)
    h_tab = nc.dram_tensor("h_tab", [NSLOT * n_cores, D], bf16,
                           addr_space="Shared")
    h_tabs = [h_tab[q * QTBL:(q + 1) * QTBL, :] for q in range(4)]

    silu = mybir.ActivationFunctionType.Silu
    copyf = mybir.ActivationFunctionType.Identity

    max_w = max(len(s) for s in stiles)

    with tile.TileContext(nc) as tc:
        lib_inst = nc.gpsimd.load_library(mlp)
        with (
            tc.tile_pool(name="persist", bufs=1) as pp,
            tc.tile_pool(name="gather", bufs=GATHER_BUFS) as gpo,
            tc.tile_pool(name="mask", bufs=16) as mpo,
            tc.tile_pool(name="agg", bufs=2) as apo,
            tc.tile_pool(name="selfp", bufs=2) as sfo,
            tc.tile_pool(name="hbuf", bufs=2) as hpo,
            tc.tile_pool(name="rx", bufs=2) as rxo,
            tc.tile_pool(name="psA", bufs=4, space="PSUM") as psa,
            tc.tile_pool(name="psT", bufs=2, space="PSUM") as pst,
            tc.tile_pool(name="psH", bufs=2, space="PSUM") as psh,
        ):
            # persistent SBUF
            idx1_sb = pp.tile([128, ep1 // 16], i16, tag="idx1", name="idx1")
            nc.sync.dma_start(idx1_sb[:], idx1_in[:])
            dstv1_sb = pp.tile([128, ep1 // 128], f32, tag="dstv1", name="dstv1")
            nc.sync.dma_start(dstv1_sb[:], dstv1_in[:])
            invv1_sb = pp.tile([128, ep1 // 128], f32, tag="invv1", name="invv1")
            nc.sync.dma_start(invv1_sb[:], invv1_in[:])
            idx2_sb = pp.tile([128, ep2 // 16], i16, tag="idx2", name="idx2")
            nc.sync.dma_start(idx2_sb[:], idx2_in[:])
            dstv2_sb = pp.tile([128, ep2 // 128], f32, tag="dstv2", name="dstv2")
            nc.sync.dma_start(dstv2_sb[:], dstv2_in[:])
            invv2_sb = pp.tile([128, ep2 // 128], f32, tag="invv2", name="invv2")
            nc.sync.dma_start(invv2_sb[:], invv2_in[:])
            iota_sb = pp.tile([128, 128], bf16)
            nc.sync.dma_start(iota_sb[:], iota_in[:])
            ident_sb = pp.tile([128, 128], bf16)
            nc.sync.dma_start(ident_sb[:], ident_in[:])
            w_sb = {}
            for nm, t in w_in.items():
                w_sb[nm] = pp.tile([128, 128], bf16, tag=nm, name=f"w_{nm}")
                nc.sync.dma_start(w_sb[nm][:], t[:])
            b_sb = {}
            for nm, t in b_in.items():
                b_sb[nm] = pp.tile([128, 1], f32, tag=nm, name=f"b_{nm}")
                nc.sync.dma_start(b_sb[nm][:], t[:])

            first_gather = [True]
            rx_count = [0]       # completed remote blocks expected so far
            if RDMA_AG:
                rx_sem = nc.alloc_semaphore("rx_sem")
                tx_sem = nc.alloc_semaphore("tx_sem")
            ht4 = h_tab[:].rearrange("(j a p) f -> p j a f", j=n_cores, p=128)

            def layer(ss, idx_sb, dstv_sb, invv_sb, src_tabs, selfT_src,
                      wl, wr, bias, is_last, ag_insts):
                """Emit one SAGE layer. Returns list of h-publish instrs."""
                gpad = ss["gpad"]; goff = ss["goff"]
                call_list = ss["call_list"]; max_slots = ss["max_slots"]
                store_insts = []
                for si_idx, tiles in enumerate(stiles):
                    w = len(tiles)
                    t0 = tiles[0]
                    # issue the stile's 4 gather calls (parallel queues)
                    gbufs = {}
                    for (q, csi, off, n) in call_list:
                        if csi != si_idx:
                            continue
                        g = gpo.tile([128, max_slots, D], bf16, tag="g")
                        slots = n // 128
                        gi = nc.gpsimd.dma_gather(
                            g[:, :slots, :], src_tabs[q][:],
                            idx_sb[:, off // 16:(off + n) // 16],
                            n, n, D, queue_num=q,
                            single_packet=False)
                        if first_gather[0]:
                            add_dep_helper(gi.ins, lib_inst.ins, sync=True,
                                           reason="lib before gather")
                            first_gather[0] = False
                        if ag_insts is not None and si_idx == 0:
                            for ag in ag_insts:
                                add_dep_helper(gi.ins, ag.ins, sync=True,
                                               reason="gather after AG")
                        gbufs[q] = (g, off)
                    # whole-stile self features: one DMA
                    selfT = sfo.tile([128, max_w * 128], bf16, tag="selfT")
                    nc.sync.dma_start(
                        selfT[:, :w * 128],
                        selfT_src[:, t0 * 128:(t0 + w) * 128])
                    # aggregation per tile -> aggT_big slices
                    aggT = apo.tile([128, max_w * 128], bf16, tag="aggT")
                    for ti, t in enumerate(tiles):
                        chunk_cols = []
                        for q in range(4):
                            npads = int(gpad[t, q])
                            if npads == 0:
                                continue
                            g, off = gbufs[q]
                            base_slot = (int(goff[t, q]) - off) // 128
                            for ci in range(npads // 128):
                                col = int(goff[t, q]) // 128 + ci
                                chunk_cols.append((g, base_slot + ci, col))
                        ps = psa.tile([128, 128], f32, tag="psagg")
                        nchunks = len(chunk_cols)
                        for j, (g, slot, col) in enumerate(chunk_cols):
                            m = mpo.tile([128, 128], bf16, tag="m")
                            nc.vector.tensor_scalar(
                                m[:], iota_sb[:],
                                dstv_sb[:, col:col + 1],
                                invv_sb[:, col:col + 1],
                                mybir.AluOpType.is_equal,
                                mybir.AluOpType.mult)
                            nc.tensor.matmul(
                                ps[:], g[:, slot, :], m[:],
                                start=(j == 0), stop=(j == nchunks - 1))
                        nc.scalar.copy(aggT[:, ti * 128:(ti + 1) * 128], ps[:])
                    # dense + epilogue in 512-wide groups
                    hT = None
                    if not is_last:
                        hT = hpo.tile([128, max_w * 128], bf16, tag="hT")
                        hr = hpo.tile([128, max_w, D], bf16, tag="hr")
                    else:
                        o = hpo.tile([128, max_w * 128], f32, tag="o")
                    for c0 in range(0, w * 128, 512):
                        c1 = min(c0 + 512, w * 128)
                        ph = psh.tile([128, 512], f32, tag="psh")
                        nc.tensor.matmul(ph[:, :c1 - c0], wl[:],
                                         aggT[:, c0:c1],
                                         start=True, stop=False)
                        nc.tensor.matmul(ph[:, :c1 - c0], wr[:],
                                         selfT[:, c0:c1],
                                         start=False, stop=True)
                        if is_last:
                            nc.scalar.activation(o[:, c0:c1], ph[:, :c1 - c0],
                                                 copyf, bias=bias[:])
                        else:
                            nc.scalar.activation(hT[:, c0:c1], ph[:, :c1 - c0],
                                                 silu, bias=bias[:])
                    if is_last:
                        nc.sync.dma_start(
                            out_t[:, t0 * 128:(t0 + w) * 128], o[:, :w * 128])
                    else:
                        nc.sync.dma_start(
                            h_selfT[:, t0 * 128:(t0 + w) * 128], hT[:, :w * 128])
                        for ti in range(w):
                            pt = pst.tile([128, 128], bf16, tag="pst")
                            nc.tensor.transpose(
                                pt[:], hT[:, ti * 128:(ti + 1) * 128],
                                ident_sb[:])
                            nc.scalar.copy(hr[:, ti, :], pt[:])
                        sins = nc.sync.dma_start(
                            h_row[t0 * 128:(t0 + w) * 128, :].rearrange(
                                "(a p) f -> p a f", p=128),
                            hr[:, :w, :])
                        store_insts.append(sins)
                return store_insts

            for _ in range(iters):
                l1_stores = layer(x_tabs, x_selfT,
                                  w_sb["w1lt"], w_sb["w1rt"], b_sb["b1"],
                                  False, None)
                ag = nc.gpsimd.collective_compute(
                    "AllGather", mybir.AluOpType.bypass,
                    replica_groups=[list(range(n_cores))],
                    ins=[h_row[:, :]],
                    outs=[h_tab[:]])
                for sins in l1_stores:
                    add_dep_helper(ag.ins, sins.ins, sync=True,
                                   reason="AG after h stores")
                layer(h_tabs, h_selfT,
                      w_sb["w2lt"], w_sb["w2rt"], b_sb["b2"],
                      True, [ag])

    nc.compile()
    return nc


# ----------------------------------------------------------------- host entry
def _prepare(edge_index):
    return _plan(edge_index, N_NODES, NC)


def _in_maps(plan, x, w1l, w1r, b1, w2l, w2r, b2):
    import ml_dtypes
    bf16 = ml_dtypes.bfloat16

    x = np.asarray(x, dtype=np.float32)
    xb = x.astype(bf16)
    nloc = plan["nloc"]
    n_cores = plan["n_cores"]
    n_nodes = plan["n_nodes"]
    slot_of = plan["slot_of"]
    gslot = (np.arange(n_nodes) // nloc) * NSLOT + slot_of

    # slot-ordered global table (shared by all cores)
    tab_full = np.zeros((NSLOT * n_cores, D), dtype=bf16)
    tab_full[gslot] = xb
    tabs = [np.ascontiguousarray(tab_full[q * QTBL:(q + 1) * QTBL])
            for q in range(4)]
    iota = np.broadcast_to(np.arange(128, dtype=np.float32),
                           (128, 128)).astype(bf16)
    ident = np.eye(128, dtype=np.float32).astype(bf16)
    wts = {
        "w1lt": np.ascontiguousarray(np.asarray(w1l, np.float32).T.astype(bf16)),
        "w1rt": np.ascontiguousarray(np.asarray(w1r, np.float32).T.astype(bf16)),
        "w2lt": np.ascontiguousarray(np.asarray(w2l, np.float32).T.astype(bf16)),
        "w2rt": np.ascontiguousarray(np.asarray(w2r, np.float32).T.astype(bf16)),
    }
    maps = []
    for k in range(n_cores):
        xsT = np.zeros((128, NSLOT), dtype=bf16)
        xsT[:, slot_of[k * nloc:(k + 1) * nloc]] = xb[k * nloc:(k + 1) * nloc].T
        m = {
            "x_selfT": xsT,
            "idxs": _wrap16(plan["idx_st"][k]),
            "dstv": _colmajor(plan["dst_st"][k]),
            "invv": _colmajor(plan["inv_st"][k]),
            "iota": iota, "ident": ident,
            "b1": np.asarray(b1, np.float32).reshape(128, 1),
            "b2": np.asarray(b2, np.float32).reshape(128, 1),
        }
        m.update(wts)
        for q in range(4):
            m[f"x_tab{q}"] = tabs[q]
        maps.append(m)
    return maps


def _run(inputs, iters=1):
    """Compile (cached) and run; returns full [N, D] output."""
    from concourse.bass_utils import run_bass_kernel_spmd

    edge_index = np.asarray(inputs["edge_index"])
    key = ("k", iters, edge_index.shape[1])
    if key not in _cache:
        plan = _prepare(edge_index)
        nc = _build(plan, iters=iters)
        _cache[key] = (plan, nc)
    plan, nc = _cache[key]
    maps = _in_maps(plan, inputs["x"], inputs["W1_l"], inputs["W1_r"],
                    inputs["b1"], inputs["W2_l"], inputs["W2_r"], inputs["b2"])
    res = run_bass_kernel_spmd(nc, maps, core_ids=list(range(plan["n_cores"])))
    nloc = plan["nloc"]
    slot_of = plan["slot_of"]
    outs = [np.asarray(res.results[k]["outT"], dtype=np.float32).T[
                slot_of[k * nloc:(k + 1) * nloc]]
            for k in range(plan["n_cores"])]
    return np.concatenate(outs, axis=0)


def kernel(**inputs) -> np.ndarray:
    return _run(inputs, iters=1)


# revision 54
# speedup vs baseline: 2.6755x; 1.5919x over previous
"""Trainium2 Bass kernel for 2-layer GraphSAGE (mean aggregation) on 8 NeuronCores.

Strategy (graph/data parallel, dst-partitioned):
  - Destination nodes sharded across 8 cores; per core, destinations are
    greedily packed (by per-quarter degree) into 100 slot-tiles of 128 so
    every (tile, src-quarter) edge group fits exactly 256 edges on every
    core (~2.4% gather padding). Host un-permutes the output.
  - Whole feature path in bf16 (gather tables, masks, weights, self
    features); PSUM accumulation stays f32.
  - Source features gathered from 4 slot-ordered quarter tables (int16
    row indices) with SWDGE dma_gather on 4 queues, double-buffered per
    super-tile of ~12 tiles.
  - Segment-mean as one-hot mask matmuls on the TensorEngine: per 128-edge
    chunk, aggT[f, d] += g_chunk.T @ mask, mask[e, d] =
    (iota[d] == dst_local[e]) * invdeg[e], one DVE tensor_scalar per chunk.
  - Dense branch feature-major in 512-wide groups: hT = W_l.T @ aggT +
    W_r.T @ selfT; self features come pre-transposed from the host
    (x_selfT) or from the layer-1 epilogue (h_selfT) - no on-device
    transpose for the self term. Layer-1 epilogue: one fused Silu
    (psum -> bf16), store h_selfT, PE-transpose to row-major h blocks.
  - Between layers: ONE AllGather of the full h shard (bf16, 26MB out)
    into the shared slot-ordered h_tab; both layers share the same edge
    streams. (An experimental pipelined remote-DMA exchange exists behind
    K_RDMA_AG=1; it does not run under this runtime's NRT shim.)
"""

import numpy as np

# ---------------------------------------------------------------- problem dims
N_NODES = 100000
N_EDGES = 800000
D = 128
NC = 8

TILE = 128            # destination-tile width
NT = 100              # dest tiles per core (underfilled for group balance)
NSLOT = NT * TILE     # padded slots per core
QTBL = NSLOT * NC // 4  # rows per quarter gather table (fits int16 idx)
GATHER_BUFS = 8
NQ = 4                # SWDGE queues

_cache = {}


def _stile_sizes(nt, n_stiles=8):
    base = nt // n_stiles
    rem = nt - base * n_stiles
    return [base + (1 if i < rem else 0) for i in range(n_stiles)]


# ------------------------------------------------------------------- host plan
def _plan(edge_index, n_nodes, n_cores):
    src = np.asarray(edge_index[0], dtype=np.int64)
    dst = np.asarray(edge_index[1], dtype=np.int64)
    E = src.shape[0]

    nloc = n_nodes // n_cores
    nt = NT

    sizes = _stile_sizes(nt)
    stiles = []
    t0 = 0
    for s in sizes:
        stiles.append(list(range(t0, t0 + s)))
        t0 += s

    deg = np.bincount(dst, minlength=n_nodes).astype(np.float64)
    invdeg = (1.0 / np.maximum(deg, 1.0)).astype(np.float32)

    # Per-core slot permutation: greedily pack destinations (desc. degree)
    # into NT tiles minimizing the max per-quarter edge load, so
    # per-(tile, quarter) group sizes stay under 256 on every core.
    # slot_of[node] = owning-core-local slot in [0, NSLOT).
    slot_of = np.empty(n_nodes, dtype=np.int64)
    # per-dst per-quarter degree (quarter = src's global-slot quarter; but the
    # quarter of a source depends on slot assignment of ITS core, which this
    # loop is computing. Use the core-of-src (pairs of cores share a quarter
    # table) as the quarter key: quarter = gslot // QTBL = core(src) // 2.
    srcq = (src // nloc) // (n_cores // 4)
    d4 = np.zeros((n_nodes, 4), dtype=np.int64)
    np.add.at(d4, (dst, srcq), 1)
    for k in range(n_cores):
        d4k = d4[k * nloc:(k + 1) * nloc]
        order_k = np.argsort(-d4k.sum(1), kind="stable")
        loads = np.zeros((nt, 4), dtype=np.int64)
        cnt = np.zeros(nt, dtype=np.int64)
        tile_k = np.empty(nloc, dtype=np.int64)
        slot_in = np.empty(nloc, dtype=np.int64)
        for idx in order_k:
            cand = (loads + d4k[idx]).max(axis=1)
            cand[cnt >= TILE] = 1 << 30
            t = int(np.argmin(cand))
            tile_k[idx] = t
            slot_in[idx] = cnt[t]
            cnt[t] += 1
            loads[t] += d4k[idx]
        slot_of[k * nloc:(k + 1) * nloc] = tile_k * TILE + slot_in

    gslot = (np.arange(n_nodes) // nloc) * NSLOT + slot_of  # global slot row

    core = dst // nloc
    core_src = src // nloc
    dslot = slot_of[dst]
    tile = dslot // TILE
    dtl = (dslot % TILE).astype(np.float32)
    inve = invdeg[dst]

    def streams(quarter, tblrow):
        gid = (core * nt + tile) * 4 + quarter
        order = np.argsort(gid, kind="stable")
        counts = np.bincount(gid, minlength=n_cores * nt * 4).reshape(
            n_cores, nt, 4)
        gmax = counts.max(axis=0)                   # [nt, 4]
        gpad = ((gmax + 127) // 128) * 128

        goff = np.zeros((nt, 4), dtype=np.int64)
        pos = 0
        call_list = []                              # (quarter, stile_idx, off, n)
        for si, tiles in enumerate(stiles):
            for q in range(4):
                call_off = pos
                for t in tiles:
                    goff[t, q] = pos
                    pos += int(gpad[t, q])
                call_list.append((q, si, call_off, pos - call_off))
        ep = pos

        idx_st = np.zeros((n_cores, ep), dtype=np.int16)
        dst_st = np.full((n_cores, ep), -1.0, dtype=np.float32)
        inv_st = np.zeros((n_cores, ep), dtype=np.float32)

        gid_s = gid[order]
        grp_start = np.searchsorted(gid_s, np.arange(n_cores * nt * 4))
        within = np.arange(E) - grp_start[gid_s]
        k_s = gid_s // (nt * 4)
        t_s = (gid_s // 4) % nt
        q_s = gid_s % 4
        put = goff[t_s, q_s] + within
        idx_st[k_s, put] = tblrow[order]
        dst_st[k_s, put] = dtl[order]
        inv_st[k_s, put] = inve[order]
        max_slots = max(n for (_, _, _, n) in call_list) // 128
        return dict(gpad=gpad, goff=goff, ep=ep, call_list=call_list,
                    idx_st=idx_st, dst_st=dst_st, inv_st=inv_st,
                    max_slots=max_slots)

    import os
    rdma = bool(int(os.environ.get("K_RDMA_AG", "0")))
    # layer 1: absolute slot-table order
    sgs = gslot[src]
    s1 = streams(sgs // QTBL, (sgs % QTBL).astype(np.int16))
    if rdma:
        # layer 2: receiver-relative core order (h_tab row block j holds
        # the shard of core (self ^ j))
        rel = (core_src ^ core) * NSLOT + slot_of[src]
        s2 = streams(rel // QTBL, (rel % QTBL).astype(np.int16))
    else:
        # collective AllGather produces absolute core order: same streams
        s2 = s1

    return dict(
        nloc=nloc, nt=nt, stiles=stiles, s1=s1, s2=s2, slot_of=slot_of,
        n_cores=n_cores, n_nodes=n_nodes,
    )


def _wrap16(stream):
    """[ep] -> [128, ep//16] wrapped-16 + replicated layout for dma_gather."""
    ep = stream.shape[0]
    w = stream.reshape(ep // 16, 16).T          # [16, ep//16]
    return np.tile(w, (8, 1))                   # [128, ep//16]


def _colmajor(stream):
    """[ep] -> [128, ep//128] with element j at [j%128, j//128]."""
    ep = stream.shape[0]
    return stream.reshape(ep // 128, 128).T.copy()


# --------------------------------------------------------------- bass builder
def _build(plan, iters=1):
    import os
    import concourse.bass as bass
    import concourse.tile as tile
    from concourse import bacc, mybir
    from concourse.library_config import mlp
    from concourse.tile_rust import add_dep_helper

    RDMA_AG = bool(int(os.environ.get("K_RDMA_AG", "0")))
    from concourse.library_config import remote_dma as remote_dma_lib

    f32 = mybir.dt.float32
    bf16 = mybir.dt.bfloat16
    i16 = mybir.dt.int16

    nt = plan["nt"]; nloc_pad = NSLOT
    stiles = plan["stiles"]
    n_cores = plan["n_cores"]
    s1, s2 = plan["s1"], plan["s2"]
    ep1, ep2 = s1["ep"], s2["ep"]

    nc = bacc.Bacc("TRN2", target_bir_lowering=False, debug=False,
                   num_swdge_queues=NQ,
                   dynamic_dma_scratch_size=24576 if RDMA_AG else 16384)
    RQ = NQ - 1          # SWDGE queue shared with (idle) gathers for RDMA

    # inputs
    x_tabs = [nc.dram_tensor(f"x_tab{q}", [QTBL, D], bf16, kind="ExternalInput")
              for q in range(4)]
    x_selfT = nc.dram_tensor("x_selfT", [128, nloc_pad], bf16, kind="ExternalInput")
    idx1_in = nc.dram_tensor("idxs1", [128, ep1 // 16], i16, kind="ExternalInput")
    dstv1_in = nc.dram_tensor("dstv1", [128, ep1 // 128], f32, kind="ExternalInput")
    invv1_in = nc.dram_tensor("invv1", [128, ep1 // 128], f32, kind="ExternalInput")
    if RDMA_AG:
        idx2_in = nc.dram_tensor("idxs2", [128, ep2 // 16], i16, kind="ExternalInput")
        dstv2_in = nc.dram_tensor("dstv2", [128, ep2 // 128], f32, kind="ExternalInput")
        invv2_in = nc.dram_tensor("invv2", [128, ep2 // 128], f32, kind="ExternalInput")
    iota_in = nc.dram_tensor("iota", [128, 128], bf16, kind="ExternalInput")
    ident_in = nc.dram_tensor("ident", [128, 128], bf16, kind="ExternalInput")
    w_in = {nm: nc.dram_tensor(nm, [128, 128], bf16, kind="ExternalInput")
            for nm in ("w1lt", "w1rt", "w2lt", "w2rt")}
    b_in = {nm: nc.dram_tensor(nm, [128, 1], f32, kind="ExternalInput")
            for nm in ("b1", "b2")}
    out_t = nc.dram_tensor("outT", [128, nloc_pad], f32, kind="ExternalOutput")

    # internal DRAM
    h_selfT = nc.dram_tensor("h_selfT", [128, nloc_pad], bf16)
    h_row = nc.dram_tensor("h_row", [nloc_pad, D], bf16)
    h_tab = nc.dram_tensor("h_tab", [NSLOT * n_cores, D], bf16,
                           addr_space="Shared")
    h_tabs = [h_tab[q * QTBL:(q + 1) * QTBL, :] for q in range(4)]

    silu = mybir.ActivationFunctionType.Silu
    copyf = mybir.ActivationFunctionType.Identity

    max_w = max(len(s) for s in stiles)

    with tile.TileContext(nc) as tc:
        lib_inst = nc.gpsimd.load_library(mlp)
        with (
            tc.tile_pool(name="persist", bufs=1) as pp,
            tc.tile_pool(name="gather", bufs=GATHER_BUFS) as gpo,
            tc.tile_pool(name="mask", bufs=16) as mpo,
            tc.tile_pool(name="agg", bufs=2) as apo,
            tc.tile_pool(name="selfp", bufs=2) as sfo,
            tc.tile_pool(name="hbuf", bufs=2) as hpo,
            tc.tile_pool(name="hrp", bufs=1 if RDMA_AG else 3) as hro,
            tc.tile_pool(name="rx", bufs=4) as rxo,
            tc.tile_pool(name="tx", bufs=2) as txo,
            tc.tile_pool(name="psA", bufs=4, space="PSUM") as psa,
            tc.tile_pool(name="psT", bufs=2, space="PSUM") as pst,
            tc.tile_pool(name="psH", bufs=2, space="PSUM") as psh,
        ):
            # persistent SBUF
            idx1_sb = pp.tile([128, ep1 // 16], i16, tag="idx1", name="idx1")
            nc.sync.dma_start(idx1_sb[:], idx1_in[:])
            dstv1_sb = pp.tile([128, ep1 // 128], f32, tag="dstv1", name="dstv1")
            nc.sync.dma_start(dstv1_sb[:], dstv1_in[:])
            invv1_sb = pp.tile([128, ep1 // 128], f32, tag="invv1", name="invv1")
            nc.sync.dma_start(invv1_sb[:], invv1_in[:])
            if RDMA_AG:
                idx2_sb = pp.tile([128, ep2 // 16], i16, tag="idx2", name="idx2")
                nc.sync.dma_start(idx2_sb[:], idx2_in[:])
                dstv2_sb = pp.tile([128, ep2 // 128], f32, tag="dstv2", name="dstv2")
                nc.sync.dma_start(dstv2_sb[:], dstv2_in[:])
                invv2_sb = pp.tile([128, ep2 // 128], f32, tag="invv2", name="invv2")
                nc.sync.dma_start(invv2_sb[:], invv2_in[:])
            else:
                idx2_sb, dstv2_sb, invv2_sb = idx1_sb, dstv1_sb, invv1_sb
            iota_sb = pp.tile([128, 128], bf16)
            nc.sync.dma_start(iota_sb[:], iota_in[:])
            ident_sb = pp.tile([128, 128], bf16)
            nc.sync.dma_start(ident_sb[:], ident_in[:])
            w_sb = {}
            for nm, t in w_in.items():
                w_sb[nm] = pp.tile([128, 128], bf16, tag=nm, name=f"w_{nm}")
                nc.sync.dma_start(w_sb[nm][:], t[:])
            b_sb = {}
            for nm, t in b_in.items():
                b_sb[nm] = pp.tile([128, 1], f32, tag=nm, name=f"b_{nm}")
                nc.sync.dma_start(b_sb[nm][:], t[:])

            first_gather = [True]
            rx_count = [0]       # expected rx_sem value so far
            post_waits = []      # (assembly inst, rx_sem threshold)
            if RDMA_AG:
                rx_sems = [nc.alloc_semaphore(f"rx_sem{j}")
                           for j in range(n_cores)]
                bar_sem = nc.alloc_semaphore("bar_sem")
                tx_sem = nc.alloc_semaphore("tx_sem")
                rxj_count = [0] * n_cores
                bar_count = [0]
            ht4 = h_tab[:].rearrange("(j a p) f -> p j a f", j=n_cores, p=128)

            def rdma_allgather(l1_gathers, hr_tiles):
                """Boundary exchange. Phase-major over destination slot j:
                every core broadcasts its h stile-blocks on slot j; receiver r
                hears from exactly one core (r ^ j) per phase, in order, into
                its j-block of h_tab (per-j arrival semaphores). Slot 0 is the
                local shard (plain DMA). GPSIMD is only needed for desc-gen,
                so the gather library is reloaded right after the last prep
                and assemblies drain on the SP queue while layer 2 starts;
                layer-2 quarter-q gathers gate only on j in {2q, 2q+1}
                assemblies. An entry barrier (sem-only broadcast) bounds
                cross-core skew before any payload flies."""
                rl1 = nc.gpsimd.load_library(remote_dma_lib)
                for gi in l1_gathers[-NQ:]:
                    add_dep_helper(rl1.ins, gi.ins, sync=True,
                                   reason="lib switch after L1 gathers")
                # entry barrier: everyone signals everyone, wait for all
                bb = nc.gpsimd.remote_sem_update_broadcast(
                    bar_sem, tx_sem,
                    rdests=[(0, jj) for jj in range(n_cores)], queue_num=RQ)
                add_dep_helper(bb.ins, rl1.ins, sync=True,
                               reason="barrier after lib switch")
                bt = nc.gpsimd.trigger_dma(count=None, queue_num=RQ)
                add_dep_helper(bt.ins, bb.ins, sync=True,
                               reason="barrier trigger")
                bar_count[0] += 16
                bw_ = nc.gpsimd.wait_ge(bar_sem, 0)
                add_dep_helper(bw_.ins, bt.ins, sync=True,
                               reason="barrier wait")
                post_waits.append((bw_, bar_sem, bar_count[0]))
                last_asm = {}          # j -> last assembly inst
                hr = hr_tiles[0]
                # j = 0: local copy of own shard
                last_asm[0] = nc.sync.dma_start(ht4[:, 0, :, :], hr[:, :, :])
                # j = 1..7: one RDMA broadcast per phase
                prev_g = bw_
                for j in range(1, n_cores):
                    rd = [None] * n_cores
                    rd[j] = (0, j)
                    rx = rxo.tile([128, nt, D], bf16, tag=f"rx{j % 2}")
                    bi = nc.gpsimd.remote_dma_broadcast(
                        rx[:, :, :], hr[:, :, :],
                        rx_sems[j], tx_sem, rdests=rd, queue_num=RQ)
                    add_dep_helper(bi.ins, prev_g.ins, sync=True,
                                   reason="bcast after barrier")
                    tg = nc.gpsimd.trigger_dma(count=None, queue_num=RQ)
                    last_tg = tg
                    rxj_count[j] += 2
                    # placeholder arrival gate; threshold patched
                    # post-scheduling (see post_waits)
                    wi = nc.sync.wait_ge(rx_sems[j], 0)
                    add_dep_helper(wi.ins, tg.ins, sync=True,
                                   reason="gate after trigger")
                    post_waits.append((wi, rx_sems[j], rxj_count[j]))
                    ai = nc.sync.dma_start(ht4[:, j, :, :], rx[:, :, :])
                    add_dep_helper(ai.ins, wi.ins, sync=True,
                                   reason="assemble after gate")
                    last_asm[j] = ai
                rl2 = nc.gpsimd.load_library(mlp)
                add_dep_helper(rl2.ins, last_tg.ins, sync=True,
                               reason="lib back after last trigger")
                # queue q gathers gate on blocks {2q, 2q+1}
                deps = {q: [last_asm[min(2 * q + 1, n_cores - 1)], rl2]
                        for q in range(NQ)}
                return deps

            def layer(ss, idx_sb, dstv_sb, invv_sb, src_tabs, selfT_src,
                      wl, wr, bias, is_last, ag_insts, hr_tiles=None):
                """Emit one SAGE layer. Returns list of h-publish instrs."""
                gpad = ss["gpad"]; goff = ss["goff"]
                call_list = ss["call_list"]; max_slots = ss["max_slots"]
                store_insts = []
                gather_insts = []
                for si_idx, tiles in enumerate(stiles):
                    w = len(tiles)
                    t0 = tiles[0]
                    # issue the stile's 4 gather calls (parallel queues)
                    gbufs = {}
                    for (q, csi, off, n) in call_list:
                        if csi != si_idx:
                            continue
                        g = gpo.tile([128, max_slots, D], bf16, tag="g")
                        slots = n // 128
                        gi = nc.gpsimd.dma_gather(
                            g[:, :slots, :], src_tabs[q][:],
                            idx_sb[:, off // 16:(off + n) // 16],
                            n, n, D, queue_num=q,
                            single_packet=False)
                        gather_insts.append(gi)
                        if first_gather[0]:
                            add_dep_helper(gi.ins, lib_inst.ins, sync=True,
                                           reason="lib before gather")
                            first_gather[0] = False
                        if ag_insts is not None and si_idx == 0:
                            for ag in ag_insts[q]:
                                add_dep_helper(gi.ins, ag.ins, sync=True,
                                               reason="gather after AG")
                        gbufs[q] = (g, off)
                    # whole-stile self features: one DMA
                    selfT = sfo.tile([128, max_w * 128], bf16, tag="selfT")
                    nc.sync.dma_start(
                        selfT[:, :w * 128],
                        selfT_src[:, t0 * 128:(t0 + w) * 128])
                    # aggregation per tile -> aggT_big slices
                    aggT = apo.tile([128, max_w * 128], bf16, tag="aggT")
                    for ti, t in enumerate(tiles):
                        chunk_cols = []
                        for q in range(4):
                            npads = int(gpad[t, q])
                            if npads == 0:
                                continue
                            g, off = gbufs[q]
                            base_slot = (int(goff[t, q]) - off) // 128
                            for ci in range(npads // 128):
                                col = int(goff[t, q]) // 128 + ci
                                chunk_cols.append((g, base_slot + ci, col))
                        ps = psa.tile([128, 128], f32, tag="psagg")
                        nchunks = len(chunk_cols)
                        for j, (g, slot, col) in enumerate(chunk_cols):
                            m = mpo.tile([128, 128], bf16, tag="m")
                            nc.vector.tensor_scalar(
                                m[:], iota_sb[:],
                                dstv_sb[:, col:col + 1],
                                invv_sb[:, col:col + 1],
                                mybir.AluOpType.is_equal,
                                mybir.AluOpType.mult)
                            nc.tensor.matmul(
                                ps[:], g[:, slot, :], m[:],
                                start=(j == 0), stop=(j == nchunks - 1))
                        nc.scalar.copy(aggT[:, ti * 128:(ti + 1) * 128], ps[:])
                    # dense + epilogue in 512-wide groups
                    hT = None
                    if is_last:
                        o_big = hpo.tile([128, max_w * 128], f32, tag="o",
                                         name="o_big")
                    else:
                        hT = hpo.tile([128, max_w * 128], bf16, tag="hT")
                    for c0 in range(0, w * 128, 512):
                        c1 = min(c0 + 512, w * 128)
                        ph = psh.tile([128, 512], f32, tag="psh")
                        nc.tensor.matmul(ph[:, :c1 - c0], wl[:],
                                         aggT[:, c0:c1],
                                         start=True, stop=False)
                        nc.tensor.matmul(ph[:, :c1 - c0], wr[:],
                                         selfT[:, c0:c1],
                                         start=False, stop=True)
                        if is_last:
                            nc.scalar.activation(o_big[:, c0:c1],
                                                 ph[:, :c1 - c0],
                                                 copyf, bias=bias[:])
                        else:
                            nc.scalar.activation(hT[:, c0:c1], ph[:, :c1 - c0],
                                                 silu, bias=bias[:])
                    if is_last:
                        nc.sync.dma_start(
                            out_t[:, t0 * 128:(t0 + w) * 128],
                            o_big[:, :w * 128])
                    else:
                        nc.sync.dma_start(
                            h_selfT[:, t0 * 128:(t0 + w) * 128], hT[:, :w * 128])
                        if RDMA_AG:
                            hr = hr_tiles[0]
                            hoff = t0
                        else:
                            hr = hro.tile([128, max_w, D], bf16, tag="hrs",
                                          name="hrs")
                            hoff = 0
                        for ti in range(w):
                            pt = pst.tile([128, 128], bf16, tag="pst")
                            nc.tensor.transpose(
                                pt[:], hT[:, ti * 128:(ti + 1) * 128],
                                ident_sb[:])
                            nc.scalar.copy(hr[:, hoff + ti, :], pt[:])
                        if not RDMA_AG:
                            sins = nc.sync.dma_start(
                                h_row[t0 * 128:(t0 + w) * 128, :].rearrange(
                                    "(a p) f -> p a f", p=128),
                                hr[:, :w, :])
                            store_insts.append(sins)
                return store_insts, gather_insts

            for _ in range(iters):
                hr_tiles = ([hro.tile([128, nt, D], bf16, tag="hrbig",
                                      name="hrbig")]
                            if RDMA_AG else [])
                l1_stores, l1_gathers = layer(
                    s1, idx1_sb, dstv1_sb, invv1_sb, x_tabs, x_selfT,
                    w_sb["w1lt"], w_sb["w1rt"], b_sb["b1"], False, None,
                    hr_tiles=hr_tiles)
                if RDMA_AG:
                    ag_deps = rdma_allgather(l1_gathers, hr_tiles)
                else:
                    ag = nc.gpsimd.collective_compute(
                        "AllGather", mybir.AluOpType.bypass,
                        replica_groups=[list(range(n_cores))],
                        ins=[h_row[:, :]],
                        outs=[h_tab[:]])
                    for sins in l1_stores:
                        add_dep_helper(ag.ins, sins.ins, sync=True,
                                       reason="AG after h stores")
                    ag_deps = {q: [ag] for q in range(NQ)}
                layer(s2, idx2_sb, dstv2_sb, invv2_sb, h_tabs, h_selfT,
                      w_sb["w2lt"], w_sb["w2rt"], b_sb["b2"], True, ag_deps)

    if RDMA_AG and not bool(int(os.environ.get("K_RX_NOWAIT", "0"))):
        for wi, sem, v in post_waits:
            done = False
            for wt in wi.ins.sync_info.on_wait:
                if wt.id == sem.num:
                    wt.wait_value = v
                    done = True
            assert done, "gate sem wait not found on gate instruction"
    nc.compile()
    return nc


# ----------------------------------------------------------------- host entry
def _prepare(edge_index):
    return _plan(edge_index, N_NODES, NC)


def _in_maps(plan, x, w1l, w1r, b1, w2l, w2r, b2):
    import ml_dtypes
    bf16 = ml_dtypes.bfloat16

    x = np.asarray(x, dtype=np.float32)
    xb = x.astype(bf16)
    nloc = plan["nloc"]
    n_cores = plan["n_cores"]
    n_nodes = plan["n_nodes"]
    slot_of = plan["slot_of"]
    gslot = (np.arange(n_nodes) // nloc) * NSLOT + slot_of

    # slot-ordered global table (shared by all cores)
    tab_full = np.zeros((NSLOT * n_cores, D), dtype=bf16)
    tab_full[gslot] = xb
    tabs = [np.ascontiguousarray(tab_full[q * QTBL:(q + 1) * QTBL])
            for q in range(4)]
    iota = np.broadcast_to(np.arange(128, dtype=np.float32),
                           (128, 128)).astype(bf16)
    ident = np.eye(128, dtype=np.float32).astype(bf16)
    wts = {
        "w1lt": np.ascontiguousarray(np.asarray(w1l, np.float32).T.astype(bf16)),
        "w1rt": np.ascontiguousarray(np.asarray(w1r, np.float32).T.astype(bf16)),
        "w2lt": np.ascontiguousarray(np.asarray(w2l, np.float32).T.astype(bf16)),
        "w2rt": np.ascontiguousarray(np.asarray(w2r, np.float32).T.astype(bf16)),
    }
    maps = []
    for k in range(n_cores):
        xsT = np.zeros((128, NSLOT), dtype=bf16)
        xsT[:, slot_of[k * nloc:(k + 1) * nloc]] = xb[k * nloc:(k + 1) * nloc].T
        m = {
            "x_selfT": xsT,
            "idxs1": _wrap16(plan["s1"]["idx_st"][k]),
        }
        if plan["s2"] is not plan["s1"]:
            m["idxs2"] = _wrap16(plan["s2"]["idx_st"][k])
            m["dstv2"] = _colmajor(plan["s2"]["dst_st"][k])
            m["invv2"] = _colmajor(plan["s2"]["inv_st"][k])
        m.update({
            "dstv1": _colmajor(plan["s1"]["dst_st"][k]),
            "invv1": _colmajor(plan["s1"]["inv_st"][k]),
            "iota": iota, "ident": ident,
            "b1": np.asarray(b1, np.float32).reshape(128, 1),
            "b2": np.asarray(b2, np.float32).reshape(128, 1),
        })
        m.update(wts)
        for q in range(4):
            m[f"x_tab{q}"] = tabs[q]
        maps.append(m)
    return maps


def _run(inputs, iters=1):
    """Compile (cached) and run; returns full [N, D] output."""
    from concourse.bass_utils import run_bass_kernel_spmd

    edge_index = np.asarray(inputs["edge_index"])
    key = ("k", iters, edge_index.shape[1])
    if key not in _cache:
        plan = _prepare(edge_index)
        nc = _build(plan, iters=iters)
        _cache[key] = (plan, nc)
    plan, nc = _cache[key]
    maps = _in_maps(plan, inputs["x"], inputs["W1_l"], inputs["W1_r"],
                    inputs["b1"], inputs["W2_l"], inputs["W2_r"], inputs["b2"])
    res = run_bass_kernel_spmd(nc, maps, core_ids=list(range(plan["n_cores"])))
    nloc = plan["nloc"]
    slot_of = plan["slot_of"]
    outs = [np.asarray(res.results[k]["outT"], dtype=np.float32).T[
                slot_of[k * nloc:(k + 1) * nloc]]
            for k in range(plan["n_cores"])]
    return np.concatenate(outs, axis=0)


def kernel(**inputs) -> np.ndarray:
    return _run(inputs, iters=1)


# revision 56
# speedup vs baseline: 1657.9871x; 619.7031x over previous
"""Trainium2 Bass kernel for 2-layer GraphSAGE (mean aggregation) on 8 NeuronCores.

Strategy (graph/data parallel, dst-partitioned):
  - Destination nodes sharded across 8 cores; per core, destinations are
    greedily packed (by per-quarter degree) into 100 slot-tiles of 128 so
    every (tile, src-quarter) edge group fits exactly 256 edges on every
    core (~2.4% gather padding). Host un-permutes the output.
  - Whole feature path in bf16 (gather tables, masks, weights, self
    features); PSUM accumulation stays f32.
  - Source features gathered from 4 slot-ordered quarter tables (int16
    row indices) with SWDGE dma_gather on 4 queues, double-buffered per
    super-tile of ~12 tiles.
  - Segment-mean as one-hot mask matmuls on the TensorEngine: per 128-edge
    chunk, aggT[f, d] += g_chunk.T @ mask, mask[e, d] =
    (iota[d] == dst_local[e]) * invdeg[e], one DVE tensor_scalar per chunk.
  - Dense branch feature-major in 512-wide groups: hT = W_l.T @ aggT +
    W_r.T @ selfT; self features come pre-transposed from the host
    (x_selfT) or from the layer-1 epilogue (h_selfT) - no on-device
    transpose for the self term. Layer-1 epilogue: one fused Silu
    (psum -> bf16), store h_selfT, PE-transpose to row-major h blocks.
  - Between layers: ONE AllGather of the full h shard (bf16, 26MB out)
    into the shared slot-ordered h_tab; both layers share the same edge
    streams. (An experimental pipelined remote-DMA exchange exists behind
    K_RDMA_AG=1; it does not run under this runtime's NRT shim.)
"""

import numpy as np

# ---------------------------------------------------------------- problem dims
N_NODES = 100000
N_EDGES = 800000
D = 128
NC = 8

TILE = 128            # destination-tile width
NT = 100              # dest tiles per core (underfilled for group balance)
NSLOT = NT * TILE     # padded slots per core
QTBL = NSLOT * NC // 4  # rows per quarter gather table (fits int16 idx)
GATHER_BUFS = 8
NQ = 4                # SWDGE queues

_cache = {}


def _stile_sizes(nt, n_stiles=8):
    base = nt // n_stiles
    rem = nt - base * n_stiles
    return [base + (1 if i < rem else 0) for i in range(n_stiles)]


# ------------------------------------------------------------------- host plan
def _plan(edge_index, n_nodes, n_cores):
    src = np.asarray(edge_index[0], dtype=np.int64)
    dst = np.asarray(edge_index[1], dtype=np.int64)
    E = src.shape[0]

    nloc = n_nodes // n_cores
    nt = NT

    sizes = _stile_sizes(nt)
    stiles = []
    t0 = 0
    for s in sizes:
        stiles.append(list(range(t0, t0 + s)))
        t0 += s

    deg = np.bincount(dst, minlength=n_nodes).astype(np.float64)
    invdeg = (1.0 / np.maximum(deg, 1.0)).astype(np.float32)

    # Per-core slot permutation: greedily pack destinations (desc. degree)
    # into NT tiles minimizing the max per-quarter edge load, so
    # per-(tile, quarter) group sizes stay under 256 on every core.
    # slot_of[node] = owning-core-local slot in [0, NSLOT).
    slot_of = np.empty(n_nodes, dtype=np.int64)
    # per-dst per-quarter degree (quarter = src's global-slot quarter; but the
    # quarter of a source depends on slot assignment of ITS core, which this
    # loop is computing. Use the core-of-src (pairs of cores share a quarter
    # table) as the quarter key: quarter = gslot // QTBL = core(src) // 2.
    srcq = (src // nloc) // (n_cores // 4)
    d4 = np.zeros((n_nodes, 4), dtype=np.int64)
    np.add.at(d4, (dst, srcq), 1)
    for k in range(n_cores):
        d4k = d4[k * nloc:(k + 1) * nloc]
        order_k = np.argsort(-d4k.sum(1), kind="stable")
        loads = np.zeros((nt, 4), dtype=np.int64)
        cnt = np.zeros(nt, dtype=np.int64)
        tile_k = np.empty(nloc, dtype=np.int64)
        slot_in = np.empty(nloc, dtype=np.int64)
        for idx in order_k:
            cand = (loads + d4k[idx]).max(axis=1)
            cand[cnt >= TILE] = 1 << 30
            t = int(np.argmin(cand))
            tile_k[idx] = t
            slot_in[idx] = cnt[t]
            cnt[t] += 1
            loads[t] += d4k[idx]
        slot_of[k * nloc:(k + 1) * nloc] = tile_k * TILE + slot_in

    gslot = (np.arange(n_nodes) // nloc) * NSLOT + slot_of  # global slot row

    core = dst // nloc
    core_src = src // nloc
    dslot = slot_of[dst]
    tile = dslot // TILE
    dtl = (dslot % TILE).astype(np.float32)
    inve = invdeg[dst]

    def streams(quarter, tblrow):
        gid = (core * nt + tile) * 4 + quarter
        order = np.argsort(gid, kind="stable")
        counts = np.bincount(gid, minlength=n_cores * nt * 4).reshape(
            n_cores, nt, 4)
        gmax = counts.max(axis=0)                   # [nt, 4]
        gpad = ((gmax + 127) // 128) * 128

        goff = np.zeros((nt, 4), dtype=np.int64)
        pos = 0
        call_list = []                              # (quarter, stile_idx, off, n)
        for si, tiles in enumerate(stiles):
            for q in range(4):
                call_off = pos
                for t in tiles:
                    goff[t, q] = pos
                    pos += int(gpad[t, q])
                call_list.append((q, si, call_off, pos - call_off))
        ep = pos

        idx_st = np.zeros((n_cores, ep), dtype=np.int16)
        dst_st = np.full((n_cores, ep), -1.0, dtype=np.float32)
        inv_st = np.zeros((n_cores, ep), dtype=np.float32)

        gid_s = gid[order]
        grp_start = np.searchsorted(gid_s, np.arange(n_cores * nt * 4))
        within = np.arange(E) - grp_start[gid_s]
        k_s = gid_s // (nt * 4)
        t_s = (gid_s // 4) % nt
        q_s = gid_s % 4
        put = goff[t_s, q_s] + within
        idx_st[k_s, put] = tblrow[order]
        dst_st[k_s, put] = dtl[order]
        inv_st[k_s, put] = inve[order]
        max_slots = max(n for (_, _, _, n) in call_list) // 128
        return dict(gpad=gpad, goff=goff, ep=ep, call_list=call_list,
                    idx_st=idx_st, dst_st=dst_st, inv_st=inv_st,
                    max_slots=max_slots)

    import os
    rdma = bool(int(os.environ.get("K_RDMA_AG", "0")))
    # layer 1: absolute slot-table order
    sgs = gslot[src]
    s1 = streams(sgs // QTBL, (sgs % QTBL).astype(np.int16))
    if rdma:
        # layer 2: receiver-relative core order (h_tab row block j holds
        # the shard of core (self ^ j))
        rel = (core_src ^ core) * NSLOT + slot_of[src]
        s2 = streams(rel // QTBL, (rel % QTBL).astype(np.int16))
    else:
        # collective AllGather produces absolute core order: same streams
        s2 = s1

    return dict(
        nloc=nloc, nt=nt, stiles=stiles, s1=s1, s2=s2, slot_of=slot_of,
        n_cores=n_cores, n_nodes=n_nodes,
    )


def _wrap16(stream):
    """[ep] -> [128, ep//16] wrapped-16 + replicated layout for dma_gather."""
    ep = stream.shape[0]
    w = stream.reshape(ep // 16, 16).T          # [16, ep//16]
    return np.tile(w, (8, 1))                   # [128, ep//16]


def _colmajor(stream):
    """[ep] -> [128, ep//128] with element j at [j%128, j//128]."""
    ep = stream.shape[0]
    return stream.reshape(ep // 128, 128).T.copy()


# --------------------------------------------------------------- bass builder
def _build(plan, iters=1):
    import os
    import concourse.bass as bass
    import concourse.tile as tile
    from concourse import bacc, mybir
    from concourse.library_config import mlp
    from concourse.tile_rust import add_dep_helper

    RDMA_AG = bool(int(os.environ.get("K_RDMA_AG", "0")))
    from concourse.library_config import remote_dma as remote_dma_lib

    f32 = mybir.dt.float32
    bf16 = mybir.dt.bfloat16
    i16 = mybir.dt.int16

    nt = plan["nt"]; nloc_pad = NSLOT
    stiles = plan["stiles"]
    n_cores = plan["n_cores"]
    s1, s2 = plan["s1"], plan["s2"]
    ep1, ep2 = s1["ep"], s2["ep"]

    nc = bacc.Bacc("TRN2", target_bir_lowering=False, debug=False,
                   num_swdge_queues=NQ,
                   dynamic_dma_scratch_size=24576 if RDMA_AG else 16384)
    RQ = NQ - 1          # SWDGE queue shared with (idle) gathers for RDMA

    # inputs
    x_tabs = [nc.dram_tensor(f"x_tab{q}", [QTBL, D], bf16, kind="ExternalInput")
              for q in range(4)]
    x_selfT = nc.dram_tensor("x_selfT", [128, nloc_pad], bf16, kind="ExternalInput")
    idx1_in = nc.dram_tensor("idxs1", [128, ep1 // 16], i16, kind="ExternalInput")
    dstv1_in = nc.dram_tensor("dstv1", [128, ep1 // 128], f32, kind="ExternalInput")
    invv1_in = nc.dram_tensor("invv1", [128, ep1 // 128], f32, kind="ExternalInput")
    if RDMA_AG:
        idx2_in = nc.dram_tensor("idxs2", [128, ep2 // 16], i16, kind="ExternalInput")
        dstv2_in = nc.dram_tensor("dstv2", [128, ep2 // 128], f32, kind="ExternalInput")
        invv2_in = nc.dram_tensor("invv2", [128, ep2 // 128], f32, kind="ExternalInput")
    iota_in = nc.dram_tensor("iota", [128, 128], bf16, kind="ExternalInput")
    ident_in = nc.dram_tensor("ident", [128, 128], bf16, kind="ExternalInput")
    w_in = {nm: nc.dram_tensor(nm, [128, 128], bf16, kind="ExternalInput")
            for nm in ("w1lt", "w1rt", "w2lt", "w2rt")}
    b_in = {nm: nc.dram_tensor(nm, [128, 1], f32, kind="ExternalInput")
            for nm in ("b1", "b2")}
    out_t = nc.dram_tensor("outT", [128, nloc_pad], f32, kind="ExternalOutput")

    # internal DRAM
    h_selfT = nc.dram_tensor("h_selfT", [128, nloc_pad], bf16)
    h_row = nc.dram_tensor("h_row", [nloc_pad, D], bf16)
    h_tab = nc.dram_tensor("h_tab", [NSLOT * n_cores, D], bf16,
                           addr_space="Shared")
    h_tabs = [h_tab[q * QTBL:(q + 1) * QTBL, :] for q in range(4)]

    silu = mybir.ActivationFunctionType.Silu
    copyf = mybir.ActivationFunctionType.Identity

    max_w = max(len(s) for s in stiles)

    with tile.TileContext(nc) as tc:
        lib_inst = nc.gpsimd.load_library(mlp)
        with (
            tc.tile_pool(name="persist", bufs=1) as pp,
            tc.tile_pool(name="gather", bufs=GATHER_BUFS) as gpo,
            tc.tile_pool(name="mask", bufs=16) as mpo,
            tc.tile_pool(name="agg", bufs=2) as apo,
            tc.tile_pool(name="selfp", bufs=2) as sfo,
            tc.tile_pool(name="hbuf", bufs=9) as hpo,
            tc.tile_pool(name="hrp", bufs=1 if RDMA_AG else 3) as hro,
            tc.tile_pool(name="rx", bufs=4) as rxo,
            tc.tile_pool(name="tx", bufs=2) as txo,
            tc.tile_pool(name="psA", bufs=4, space="PSUM") as psa,
            tc.tile_pool(name="psT", bufs=2, space="PSUM") as pst,
            tc.tile_pool(name="psH", bufs=2, space="PSUM") as psh,
        ):
            # persistent SBUF
            idx1_sb = pp.tile([128, ep1 // 16], i16, tag="idx1", name="idx1")
            nc.sync.dma_start(idx1_sb[:], idx1_in[:])
            dstv1_sb = pp.tile([128, ep1 // 128], f32, tag="dstv1", name="dstv1")
            nc.sync.dma_start(dstv1_sb[:], dstv1_in[:])
            invv1_sb = pp.tile([128, ep1 // 128], f32, tag="invv1", name="invv1")
            nc.sync.dma_start(invv1_sb[:], invv1_in[:])
            if RDMA_AG:
                idx2_sb = pp.tile([128, ep2 // 16], i16, tag="idx2", name="idx2")
                nc.sync.dma_start(idx2_sb[:], idx2_in[:])
                dstv2_sb = pp.tile([128, ep2 // 128], f32, tag="dstv2", name="dstv2")
                nc.sync.dma_start(dstv2_sb[:], dstv2_in[:])
                invv2_sb = pp.tile([128, ep2 // 128], f32, tag="invv2", name="invv2")
                nc.sync.dma_start(invv2_sb[:], invv2_in[:])
            else:
                idx2_sb, dstv2_sb, invv2_sb = idx1_sb, dstv1_sb, invv1_sb
            iota_sb = pp.tile([128, 128], bf16)
            nc.sync.dma_start(iota_sb[:], iota_in[:])
            ident_sb = pp.tile([128, 128], bf16)
            nc.sync.dma_start(ident_sb[:], ident_in[:])
            w_sb = {}
            for nm, t in w_in.items():
                w_sb[nm] = pp.tile([128, 128], bf16, tag=nm, name=f"w_{nm}")
                nc.sync.dma_start(w_sb[nm][:], t[:])
            b_sb = {}
            for nm, t in b_in.items():
                b_sb[nm] = pp.tile([128, 1], f32, tag=nm, name=f"b_{nm}")
                nc.sync.dma_start(b_sb[nm][:], t[:])

            first_gather = [True]
            rx_count = [0]       # expected rx_sem value so far
            post_waits = []      # (assembly inst, rx_sem threshold)
            if RDMA_AG:
                rx_sems = [nc.alloc_semaphore(f"rx_sem{j}")
                           for j in range(n_cores)]
                bar_sem = nc.alloc_semaphore("bar_sem")
                tx_sem = nc.alloc_semaphore("tx_sem")
                rxj_count = [0] * n_cores
                bar_count = [0]
            ht4 = h_tab[:].rearrange("(j a p) f -> p j a f", j=n_cores, p=128)

            def rdma_allgather(l1_gathers, hr_tiles):
                """Boundary exchange. Phase-major over destination slot j:
                every core broadcasts its h stile-blocks on slot j; receiver r
                hears from exactly one core (r ^ j) per phase, in order, into
                its j-block of h_tab (per-j arrival semaphores). Slot 0 is the
                local shard (plain DMA). GPSIMD is only needed for desc-gen,
                so the gather library is reloaded right after the last prep
                and assemblies drain on the SP queue while layer 2 starts;
                layer-2 quarter-q gathers gate only on j in {2q, 2q+1}
                assemblies. An entry barrier (sem-only broadcast) bounds
                cross-core skew before any payload flies."""
                rl1 = nc.gpsimd.load_library(remote_dma_lib)
                for gi in l1_gathers[-NQ:]:
                    add_dep_helper(rl1.ins, gi.ins, sync=True,
                                   reason="lib switch after L1 gathers")
                # entry barrier: everyone signals everyone, wait for all
                bb = nc.gpsimd.remote_sem_update_broadcast(
                    bar_sem, tx_sem,
                    rdests=[(0, jj) for jj in range(n_cores)], queue_num=RQ)
                add_dep_helper(bb.ins, rl1.ins, sync=True,
                               reason="barrier after lib switch")
                bt = nc.gpsimd.trigger_dma(count=None, queue_num=RQ)
                add_dep_helper(bt.ins, bb.ins, sync=True,
                               reason="barrier trigger")
                bar_count[0] += 16
                bw_ = nc.gpsimd.wait_ge(bar_sem, 0)
                add_dep_helper(bw_.ins, bt.ins, sync=True,
                               reason="barrier wait")
                post_waits.append((bw_, bar_sem, bar_count[0]))
                last_asm = {}          # j -> last assembly inst
                hr = hr_tiles[0]
                # j = 0: local copy of own shard
                last_asm[0] = nc.sync.dma_start(ht4[:, 0, :, :], hr[:, :, :])
                # j = 1..7: one RDMA broadcast per phase
                prev_g = bw_
                for j in range(1, n_cores):
                    rd = [None] * n_cores
                    rd[j] = (0, j)
                    rx = rxo.tile([128, nt, D], bf16, tag=f"rx{j % 2}")
                    bi = nc.gpsimd.remote_dma_broadcast(
                        rx[:, :, :], hr[:, :, :],
                        rx_sems[j], tx_sem, rdests=rd, queue_num=RQ)
                    add_dep_helper(bi.ins, prev_g.ins, sync=True,
                                   reason="bcast after barrier")
                    tg = nc.gpsimd.trigger_dma(count=None, queue_num=RQ)
                    last_tg = tg
                    rxj_count[j] += 2
                    # placeholder arrival gate; threshold patched
                    # post-scheduling (see post_waits)
                    wi = nc.sync.wait_ge(rx_sems[j], 0)
                    add_dep_helper(wi.ins, tg.ins, sync=True,
                                   reason="gate after trigger")
                    post_waits.append((wi, rx_sems[j], rxj_count[j]))
                    ai = nc.sync.dma_start(ht4[:, j, :, :], rx[:, :, :])
                    add_dep_helper(ai.ins, wi.ins, sync=True,
                                   reason="assemble after gate")
                    last_asm[j] = ai
                rl2 = nc.gpsimd.load_library(mlp)
                add_dep_helper(rl2.ins, last_tg.ins, sync=True,
                               reason="lib back after last trigger")
                # queue q gathers gate on blocks {2q, 2q+1}
                deps = {q: [last_asm[min(2 * q + 1, n_cores - 1)], rl2]
                        for q in range(NQ)}
                return deps

            def layer(ss, idx_sb, dstv_sb, invv_sb, src_tabs, selfT_src,
                      wl, wr, bias, is_last, ag_insts, hr_tiles=None,
                      hT_tiles=None, self_sb=None):
                """Emit one SAGE layer. Returns list of h-publish instrs."""
                gpad = ss["gpad"]; goff = ss["goff"]
                call_list = ss["call_list"]; max_slots = ss["max_slots"]
                store_insts = []
                gather_insts = []
                for si_idx, tiles in enumerate(stiles):
                    w = len(tiles)
                    t0 = tiles[0]
                    # issue the stile's 4 gather calls (parallel queues)
                    gbufs = {}
                    for (q, csi, off, n) in call_list:
                        if csi != si_idx:
                            continue
                        g = gpo.tile([128, max_slots, D], bf16, tag="g")
                        slots = n // 128
                        gi = nc.gpsimd.dma_gather(
                            g[:, :slots, :], src_tabs[q][:],
                            idx_sb[:, off // 16:(off + n) // 16],
                            n, n, D, queue_num=q,
                            single_packet=False)
                        gather_insts.append(gi)
                        if first_gather[0]:
                            add_dep_helper(gi.ins, lib_inst.ins, sync=True,
                                           reason="lib before gather")
                            first_gather[0] = False
                        if ag_insts is not None and si_idx == 0:
                            for ag in ag_insts[q]:
                                add_dep_helper(gi.ins, ag.ins, sync=True,
                                               reason="gather after AG")
                        gbufs[q] = (g, off)
                    # whole-stile self features: SBUF-resident (layer 2)
                    # or one DMA (layer 1)
                    if self_sb is not None:
                        selfT = self_sb[si_idx]
                    else:
                        selfT = sfo.tile([128, max_w * 128], bf16, tag="selfT")
                        nc.sync.dma_start(
                            selfT[:, :w * 128],
                            selfT_src[:, t0 * 128:(t0 + w) * 128])
                    # aggregation per tile -> aggT_big slices
                    aggT = apo.tile([128, max_w * 128], bf16, tag="aggT")
                    for ti, t in enumerate(tiles):
                        chunk_cols = []
                        for q in range(4):
                            npads = int(gpad[t, q])
                            if npads == 0:
                                continue
                            g, off = gbufs[q]
                            base_slot = (int(goff[t, q]) - off) // 128
                            for ci in range(npads // 128):
                                col = int(goff[t, q]) // 128 + ci
                                chunk_cols.append((g, base_slot + ci, col))
                        ps = psa.tile([128, 128], f32, tag="psagg")
                        nchunks = len(chunk_cols)
                        for j, (g, slot, col) in enumerate(chunk_cols):
                            m = mpo.tile([128, 128], bf16, tag="m")
                            nc.vector.tensor_scalar(
                                m[:], iota_sb[:],
                                dstv_sb[:, col:col + 1],
                                invv_sb[:, col:col + 1],
                                mybir.AluOpType.is_equal,
                                mybir.AluOpType.mult)
                            nc.tensor.matmul(
                                ps[:], g[:, slot, :], m[:],
                                start=(j == 0), stop=(j == nchunks - 1))
                        nc.scalar.copy(aggT[:, ti * 128:(ti + 1) * 128], ps[:])
                    # dense + epilogue in 512-wide groups
                    hT = None
                    if is_last:
                        o_big = hpo.tile([128, max_w * 128], f32, tag="o",
                                         name="o_big")
                    else:
                        hT = hpo.tile([128, max_w * 128], bf16, tag="hT")
                        if hT_tiles is not None:
                            hT_tiles.append(hT)
                    for c0 in range(0, w * 128, 512):
                        c1 = min(c0 + 512, w * 128)
                        ph = psh.tile([128, 512], f32, tag="psh")
                        nc.tensor.matmul(ph[:, :c1 - c0], wl[:],
                                         aggT[:, c0:c1],
                                         start=True, stop=False)
                        nc.tensor.matmul(ph[:, :c1 - c0], wr[:],
                                         selfT[:, c0:c1],
                                         start=False, stop=True)
                        if is_last:
                            nc.scalar.activation(o_big[:, c0:c1],
                                                 ph[:, :c1 - c0],
                                                 copyf, bias=bias[:])
                        else:
                            nc.scalar.activation(hT[:, c0:c1], ph[:, :c1 - c0],
                                                 silu, bias=bias[:])
                    if is_last:
                        nc.sync.dma_start(
                            out_t[:, t0 * 128:(t0 + w) * 128],
                            o_big[:, :w * 128])
                    else:
                        if RDMA_AG:
                            hr = hr_tiles[0]
                            hoff = t0
                        else:
                            hr = hro.tile([128, max_w, D], bf16, tag="hrs",
                                          name="hrs")
                            hoff = 0
                        for ti in range(w):
                            pt = pst.tile([128, 128], bf16, tag="pst")
                            nc.tensor.transpose(
                                pt[:], hT[:, ti * 128:(ti + 1) * 128],
                                ident_sb[:])
                            nc.scalar.copy(hr[:, hoff + ti, :], pt[:])
                        if not RDMA_AG:
                            sins = nc.sync.dma_start(
                                h_row[t0 * 128:(t0 + w) * 128, :].rearrange(
                                    "(a p) f -> p a f", p=128),
                                hr[:, :w, :])
                            store_insts.append(sins)
                return store_insts, gather_insts

            for _ in range(iters):
                hr_tiles = ([hro.tile([128, nt, D], bf16, tag="hrbig",
                                      name="hrbig")]
                            if RDMA_AG else [])
                hT_tiles = []
                l1_stores, l1_gathers = layer(
                    s1, idx1_sb, dstv1_sb, invv1_sb, x_tabs, x_selfT,
                    w_sb["w1lt"], w_sb["w1rt"], b_sb["b1"], False, None,
                    hr_tiles=hr_tiles, hT_tiles=hT_tiles)
                if RDMA_AG:
                    ag_deps = rdma_allgather(l1_gathers, hr_tiles)
                else:
                    ag = nc.gpsimd.collective_compute(
                        "AllGather", mybir.AluOpType.bypass,
                        replica_groups=[list(range(n_cores))],
                        ins=[h_row[:, :]],
                        outs=[h_tab[:]])
                    for sins in l1_stores:
                        add_dep_helper(ag.ins, sins.ins, sync=True,
                                       reason="AG after h stores")
                    ag_deps = {q: [ag] for q in range(NQ)}
                layer(s2, idx2_sb, dstv2_sb, invv2_sb, h_tabs, None,
                      w_sb["w2lt"], w_sb["w2rt"], b_sb["b2"], True, ag_deps,
                      self_sb=hT_tiles)

    if RDMA_AG and not bool(int(os.environ.get("K_RX_NOWAIT", "0"))):
        for wi, sem, v in post_waits:
            done = False
            for wt in wi.ins.sync_info.on_wait:
                if wt.id == sem.num:
                    wt.wait_value = v
                    done = True
            assert done, "gate sem wait not found on gate instruction"
    nc.compile()
    return nc


# ----------------------------------------------------------------- host entry
def _prepare(edge_index):
    return _plan(edge_index, N_NODES, NC)


def _in_maps(plan, x, w1l, w1r, b1, w2l, w2r, b2):
    import ml_dtypes
    bf16 = ml_dtypes.bfloat16

    x = np.asarray(x, dtype=np.float32)
    xb = x.astype(bf16)
    nloc = plan["nloc"]
    n_cores = plan["n_cores"]
    n_nodes = plan["n_nodes"]
    slot_of = plan["slot_of"]
    gslot = (np.arange(n_nodes) // nloc) * NSLOT + slot_of

    # slot-ordered global table (shared by all cores)
    tab_full = np.zeros((NSLOT * n_cores, D), dtype=bf16)
    tab_full[gslot] = xb
    tabs = [np.ascontiguousarray(tab_full[q * QTBL:(q + 1) * QTBL])
            for q in range(4)]
    iota = np.broadcast_to(np.arange(128, dtype=np.float32),
                           (128, 128)).astype(bf16)
    ident = np.eye(128, dtype=np.float32).astype(bf16)
    wts = {
        "w1lt": np.ascontiguousarray(np.asarray(w1l, np.float32).T.astype(bf16)),
        "w1rt": np.ascontiguousarray(np.asarray(w1r, np.float32).T.astype(bf16)),
        "w2lt": np.ascontiguousarray(np.asarray(w2l, np.float32).T.astype(bf16)),
        "w2rt": np.ascontiguousarray(np.asarray(w2r, np.float32).T.astype(bf16)),
    }
    maps = []
    for k in range(n_cores):
        xsT = np.zeros((128, NSLOT), dtype=bf16)
        xsT[:, slot_of[k * nloc:(k + 1) * nloc]] = xb[k * nloc:(k + 1) * nloc].T
        m = {
            "x_selfT": xsT,
            "idxs1": _wrap16(plan["s1"]["idx_st"][k]),
        }
        if plan["s2"] is not plan["s1"]:
            m["idxs2"] = _wrap16(plan["s2"]["idx_st"][k])
            m["dstv2"] = _colmajor(plan["s2"]["dst_st"][k])
            m["invv2"] = _colmajor(plan["s2"]["inv_st"][k])
        m.update({
            "dstv1": _colmajor(plan["s1"]["dst_st"][k]),
            "invv1": _colmajor(plan["s1"]["inv_st"][k]),
            "iota": iota, "ident": ident,
            "b1": np.asarray(b1, np.float32).reshape(128, 1),
            "b2": np.asarray(b2, np.float32).reshape(128, 1),
        })
        m.update(wts)
        for q in range(4):
            m[f"x_tab{q}"] = tabs[q]
        maps.append(m)
    return maps


def _run(inputs, iters=1):
    """Compile (cached) and run; returns full [N, D] output."""
    from concourse.bass_utils import run_bass_kernel_spmd

    edge_index = np.asarray(inputs["edge_index"])
    key = ("k", iters, edge_index.shape[1])
    if key not in _cache:
        plan = _prepare(edge_index)
        nc = _build(plan, iters=iters)
        _cache[key] = (plan, nc)
    plan, nc = _cache[key]
    maps = _in_maps(plan, inputs["x"], inputs["W1_l"], inputs["W1_r"],
                    inputs["b1"], inputs["W2_l"], inputs["W2_r"], inputs["b2"])
    res = run_bass_kernel_spmd(nc, maps, core_ids=list(range(plan["n_cores"])))
    nloc = plan["nloc"]
    slot_of = plan["slot_of"]
    outs = [np.asarray(res.results[k]["outT"], dtype=np.float32).T[
                slot_of[k * nloc:(k + 1) * nloc]]
            for k in range(plan["n_cores"])]
    return np.concatenate(outs, axis=0)


def kernel(**inputs) -> np.ndarray:
    return _run(inputs, iters=1)


# revision 60
# speedup vs baseline: 1677.5825x; 1.0118x over previous
"""Trainium2 Bass kernel for 2-layer GraphSAGE (mean aggregation) on 8 NeuronCores.

Strategy (graph/data parallel, dst-partitioned):
  - Destination nodes sharded across 8 cores; per core, destinations are
    greedily packed (by per-quarter degree) into 100 slot-tiles of 128 so
    every (tile, src-quarter) edge group fits exactly 256 edges on every
    core (~2.4% gather padding). Host un-permutes the output.
  - Whole feature path in bf16 (gather tables, masks, weights, self
    features); PSUM accumulation stays f32.
  - Source features gathered from 4 slot-ordered quarter tables (int16
    row indices) with SWDGE dma_gather on 4 queues, double-buffered per
    super-tile of ~12 tiles.
  - Segment-mean as one-hot mask matmuls on the TensorEngine: per 128-edge
    chunk, aggT[f, d] += g_chunk.T @ mask, mask[e, d] =
    (iota[d] == dst_local[e]) * invdeg[e], one DVE tensor_scalar per chunk.
  - Dense branch feature-major in 512-wide groups: hT = W_l.T @ aggT +
    W_r.T @ selfT; self features come pre-transposed from the host
    (x_selfT) or from the layer-1 epilogue (h_selfT) - no on-device
    transpose for the self term. Layer-1 epilogue: one fused Silu
    (psum -> bf16), store h_selfT, PE-transpose to row-major h blocks.
  - Between layers: ONE AllGather of the full h shard (bf16, 26MB out)
    into the shared slot-ordered h_tab; both layers share the same edge
    streams. (An experimental pipelined remote-DMA exchange exists behind
    K_RDMA_AG=1; it does not run under this runtime's NRT shim.)
"""

import numpy as np

# ---------------------------------------------------------------- problem dims
N_NODES = 100000
N_EDGES = 800000
D = 128
NC = 8

TILE = 128            # destination-tile width
NT = 100              # dest tiles per core (underfilled for group balance)
NSLOT = NT * TILE     # padded slots per core
QTBL = NSLOT * NC // 4  # rows per quarter gather table (fits int16 idx)
GATHER_BUFS = 8
NQ = 4                # SWDGE queues

_cache = {}


def _stile_sizes(nt, n_stiles=8):
    base = nt // n_stiles
    rem = nt - base * n_stiles
    return [base + (1 if i < rem else 0) for i in range(n_stiles)]


# ------------------------------------------------------------------- host plan
def _plan(edge_index, n_nodes, n_cores):
    src = np.asarray(edge_index[0], dtype=np.int64)
    dst = np.asarray(edge_index[1], dtype=np.int64)
    E = src.shape[0]

    nloc = n_nodes // n_cores
    nt = NT

    sizes = _stile_sizes(nt)
    stiles = []
    t0 = 0
    for s in sizes:
        stiles.append(list(range(t0, t0 + s)))
        t0 += s

    deg = np.bincount(dst, minlength=n_nodes).astype(np.float64)
    invdeg = (1.0 / np.maximum(deg, 1.0)).astype(np.float32)

    # Per-core slot permutation: greedily pack destinations (desc. degree)
    # into NT tiles minimizing the max per-quarter edge load, so
    # per-(tile, quarter) group sizes stay under 256 on every core.
    # slot_of[node] = owning-core-local slot in [0, NSLOT).
    slot_of = np.empty(n_nodes, dtype=np.int64)
    # per-dst per-quarter degree (quarter = src's global-slot quarter; but the
    # quarter of a source depends on slot assignment of ITS core, which this
    # loop is computing. Use the core-of-src (pairs of cores share a quarter
    # table) as the quarter key: quarter = gslot // QTBL = core(src) // 2.
    srcq = (src // nloc) // (n_cores // 4)
    d4 = np.zeros((n_nodes, 4), dtype=np.int64)
    np.add.at(d4, (dst, srcq), 1)
    for k in range(n_cores):
        d4k = d4[k * nloc:(k + 1) * nloc]
        order_k = np.argsort(-d4k.sum(1), kind="stable")
        loads = np.zeros((nt, 4), dtype=np.int64)
        cnt = np.zeros(nt, dtype=np.int64)
        tile_k = np.empty(nloc, dtype=np.int64)
        slot_in = np.empty(nloc, dtype=np.int64)
        for idx in order_k:
            cand = (loads + d4k[idx]).max(axis=1)
            cand[cnt >= TILE] = 1 << 30
            t = int(np.argmin(cand))
            tile_k[idx] = t
            slot_in[idx] = cnt[t]
            cnt[t] += 1
            loads[t] += d4k[idx]
        slot_of[k * nloc:(k + 1) * nloc] = tile_k * TILE + slot_in

    gslot = (np.arange(n_nodes) // nloc) * NSLOT + slot_of  # global slot row

    core = dst // nloc
    core_src = src // nloc
    dslot = slot_of[dst]
    tile = dslot // TILE
    dtl = (dslot % TILE).astype(np.float32)
    inve = invdeg[dst]

    def streams(quarter, tblrow):
        gid = (core * nt + tile) * 4 + quarter
        order = np.argsort(gid, kind="stable")
        counts = np.bincount(gid, minlength=n_cores * nt * 4).reshape(
            n_cores, nt, 4)
        gmax = counts.max(axis=0)                   # [nt, 4]
        gpad = ((gmax + 127) // 128) * 128

        goff = np.zeros((nt, 4), dtype=np.int64)
        pos = 0
        call_list = []                              # (quarter, stile_idx, off, n)
        for si, tiles in enumerate(stiles):
            for q in range(4):
                call_off = pos
                for t in tiles:
                    goff[t, q] = pos
                    pos += int(gpad[t, q])
                call_list.append((q, si, call_off, pos - call_off))
        ep = pos

        idx_st = np.zeros((n_cores, ep), dtype=np.int16)
        dst_st = np.full((n_cores, ep), -1.0, dtype=np.float32)
        inv_st = np.zeros((n_cores, ep), dtype=np.float32)

        gid_s = gid[order]
        grp_start = np.searchsorted(gid_s, np.arange(n_cores * nt * 4))
        within = np.arange(E) - grp_start[gid_s]
        k_s = gid_s // (nt * 4)
        t_s = (gid_s // 4) % nt
        q_s = gid_s % 4
        put = goff[t_s, q_s] + within
        idx_st[k_s, put] = tblrow[order]
        dst_st[k_s, put] = dtl[order]
        inv_st[k_s, put] = inve[order]
        max_slots = max(n for (_, _, _, n) in call_list) // 128
        return dict(gpad=gpad, goff=goff, ep=ep, call_list=call_list,
                    idx_st=idx_st, dst_st=dst_st, inv_st=inv_st,
                    max_slots=max_slots)

    import os
    rdma = bool(int(os.environ.get("K_RDMA_AG", "0")))
    # layer 1: absolute slot-table order
    sgs = gslot[src]
    s1 = streams(sgs // QTBL, (sgs % QTBL).astype(np.int16))
    if rdma:
        # layer 2: receiver-relative core order (h_tab row block j holds
        # the shard of core (self ^ j))
        rel = (core_src ^ core) * NSLOT + slot_of[src]
        s2 = streams(rel // QTBL, (rel % QTBL).astype(np.int16))
    else:
        # collective AllGather produces absolute core order: same streams
        s2 = s1

    return dict(
        nloc=nloc, nt=nt, stiles=stiles, s1=s1, s2=s2, slot_of=slot_of,
        n_cores=n_cores, n_nodes=n_nodes,
    )


def _wrap16(stream):
    """[ep] -> [128, ep//16] wrapped-16 + replicated layout for dma_gather."""
    ep = stream.shape[0]
    w = stream.reshape(ep // 16, 16).T          # [16, ep//16]
    return np.tile(w, (8, 1))                   # [128, ep//16]


def _colmajor(stream):
    """[ep] -> [128, ep//128] with element j at [j%128, j//128]."""
    ep = stream.shape[0]
    return stream.reshape(ep // 128, 128).T.copy()


# --------------------------------------------------------------- bass builder
def _build(plan, iters=1):
    import os
    import concourse.bass as bass
    import concourse.tile as tile
    from concourse import bacc, mybir
    from concourse.library_config import mlp
    from concourse.tile_rust import add_dep_helper

    RDMA_AG = bool(int(os.environ.get("K_RDMA_AG", "0")))
    from concourse.library_config import remote_dma as remote_dma_lib

    f32 = mybir.dt.float32
    bf16 = mybir.dt.bfloat16
    i16 = mybir.dt.int16

    nt = plan["nt"]; nloc_pad = NSLOT
    stiles = plan["stiles"]
    n_cores = plan["n_cores"]
    s1, s2 = plan["s1"], plan["s2"]
    ep1, ep2 = s1["ep"], s2["ep"]

    nc = bacc.Bacc("TRN2", target_bir_lowering=False, debug=False,
                   num_swdge_queues=NQ,
                   dynamic_dma_scratch_size=24576 if RDMA_AG else 16384)
    RQ = NQ - 1          # SWDGE queue shared with (idle) gathers for RDMA

    # inputs
    x_tabs = [nc.dram_tensor(f"x_tab{q}", [QTBL, D], bf16, kind="ExternalInput")
              for q in range(4)]
    x_selfT = nc.dram_tensor("x_selfT", [128, nloc_pad], bf16, kind="ExternalInput")
    idx1_in = nc.dram_tensor("idxs1", [128, ep1 // 16], i16, kind="ExternalInput")
    dstv1_in = nc.dram_tensor("dstv1", [128, ep1 // 128], f32, kind="ExternalInput")
    invv1_in = nc.dram_tensor("invv1", [128, ep1 // 128], f32, kind="ExternalInput")
    if RDMA_AG:
        idx2_in = nc.dram_tensor("idxs2", [128, ep2 // 16], i16, kind="ExternalInput")
        dstv2_in = nc.dram_tensor("dstv2", [128, ep2 // 128], f32, kind="ExternalInput")
        invv2_in = nc.dram_tensor("invv2", [128, ep2 // 128], f32, kind="ExternalInput")
    iota_in = nc.dram_tensor("iota", [128, 128], bf16, kind="ExternalInput")
    ident_in = nc.dram_tensor("ident", [128, 128], bf16, kind="ExternalInput")
    w_in = {nm: nc.dram_tensor(nm, [128, 128], bf16, kind="ExternalInput")
            for nm in ("w1lt", "w1rt", "w2lt", "w2rt")}
    b_in = {nm: nc.dram_tensor(nm, [128, 1], f32, kind="ExternalInput")
            for nm in ("b1", "b2")}
    out_t = nc.dram_tensor("outT", [128, nloc_pad], bf16, kind="ExternalOutput")

    # internal DRAM
    h_selfT = nc.dram_tensor("h_selfT", [128, nloc_pad], bf16)
    h_row = nc.dram_tensor("h_row", [nloc_pad, D], bf16)
    h_tab = nc.dram_tensor("h_tab", [NSLOT * n_cores, D], bf16,
                           addr_space="Shared")
    h_tabs = [h_tab[q * QTBL:(q + 1) * QTBL, :] for q in range(4)]

    silu = mybir.ActivationFunctionType.Silu
    copyf = mybir.ActivationFunctionType.Identity

    max_w = max(len(s) for s in stiles)

    with tile.TileContext(nc) as tc:
        lib_inst = nc.gpsimd.load_library(mlp)
        with (
            tc.tile_pool(name="persist", bufs=1) as pp,
            tc.tile_pool(name="gather", bufs=GATHER_BUFS) as gpo,
            tc.tile_pool(name="mask", bufs=16) as mpo,
            tc.tile_pool(name="agg", bufs=2) as apo,
            tc.tile_pool(name="selfp", bufs=2) as sfo,
            tc.tile_pool(name="hbuf", bufs=9) as hpo,
            tc.tile_pool(name="hrp", bufs=1 if RDMA_AG else 3) as hro,
            tc.tile_pool(name="rx", bufs=4) as rxo,
            tc.tile_pool(name="tx", bufs=2) as txo,
            tc.tile_pool(name="psA", bufs=4, space="PSUM") as psa,
            tc.tile_pool(name="psT", bufs=2, space="PSUM") as pst,
            tc.tile_pool(name="psH", bufs=2, space="PSUM") as psh,
        ):
            # persistent SBUF
            idx1_sb = pp.tile([128, ep1 // 16], i16, tag="idx1", name="idx1")
            nc.sync.dma_start(idx1_sb[:], idx1_in[:])
            dstv1_sb = pp.tile([128, ep1 // 128], f32, tag="dstv1", name="dstv1")
            nc.sync.dma_start(dstv1_sb[:], dstv1_in[:])
            invv1_sb = pp.tile([128, ep1 // 128], f32, tag="invv1", name="invv1")
            nc.sync.dma_start(invv1_sb[:], invv1_in[:])
            if RDMA_AG:
                idx2_sb = pp.tile([128, ep2 // 16], i16, tag="idx2", name="idx2")
                nc.sync.dma_start(idx2_sb[:], idx2_in[:])
                dstv2_sb = pp.tile([128, ep2 // 128], f32, tag="dstv2", name="dstv2")
                nc.sync.dma_start(dstv2_sb[:], dstv2_in[:])
                invv2_sb = pp.tile([128, ep2 // 128], f32, tag="invv2", name="invv2")
                nc.sync.dma_start(invv2_sb[:], invv2_in[:])
            else:
                idx2_sb, dstv2_sb, invv2_sb = idx1_sb, dstv1_sb, invv1_sb
            iota_sb = pp.tile([128, 128], bf16)
            nc.sync.dma_start(iota_sb[:], iota_in[:])
            ident_sb = pp.tile([128, 128], bf16)
            nc.sync.dma_start(ident_sb[:], ident_in[:])
            w_sb = {}
            for nm, t in w_in.items():
                w_sb[nm] = pp.tile([128, 128], bf16, tag=nm, name=f"w_{nm}")
                nc.sync.dma_start(w_sb[nm][:], t[:])
            b_sb = {}
            for nm, t in b_in.items():
                b_sb[nm] = pp.tile([128, 1], f32, tag=nm, name=f"b_{nm}")
                nc.sync.dma_start(b_sb[nm][:], t[:])

            first_gather = [True]
            rx_count = [0]       # expected rx_sem value so far
            post_waits = []      # (assembly inst, rx_sem threshold)
            if RDMA_AG:
                rx_sems = [nc.alloc_semaphore(f"rx_sem{j}")
                           for j in range(n_cores)]
                bar_sem = nc.alloc_semaphore("bar_sem")
                tx_sem = nc.alloc_semaphore("tx_sem")
                rxj_count = [0] * n_cores
                bar_count = [0]
            ht4 = h_tab[:].rearrange("(j a p) f -> p j a f", j=n_cores, p=128)

            def rdma_allgather(l1_gathers, hr_tiles):
                """Boundary exchange. Phase-major over destination slot j:
                every core broadcasts its h stile-blocks on slot j; receiver r
                hears from exactly one core (r ^ j) per phase, in order, into
                its j-block of h_tab (per-j arrival semaphores). Slot 0 is the
                local shard (plain DMA). GPSIMD is only needed for desc-gen,
                so the gather library is reloaded right after the last prep
                and assemblies drain on the SP queue while layer 2 starts;
                layer-2 quarter-q gathers gate only on j in {2q, 2q+1}
                assemblies. An entry barrier (sem-only broadcast) bounds
                cross-core skew before any payload flies."""
                rl1 = nc.gpsimd.load_library(remote_dma_lib)
                for gi in l1_gathers[-NQ:]:
                    add_dep_helper(rl1.ins, gi.ins, sync=True,
                                   reason="lib switch after L1 gathers")
                # entry barrier: everyone signals everyone, wait for all
                bb = nc.gpsimd.remote_sem_update_broadcast(
                    bar_sem, tx_sem,
                    rdests=[(0, jj) for jj in range(n_cores)], queue_num=RQ)
                add_dep_helper(bb.ins, rl1.ins, sync=True,
                               reason="barrier after lib switch")
                bt = nc.gpsimd.trigger_dma(count=None, queue_num=RQ)
                add_dep_helper(bt.ins, bb.ins, sync=True,
                               reason="barrier trigger")
                bar_count[0] += 16
                bw_ = nc.gpsimd.wait_ge(bar_sem, 0)
                add_dep_helper(bw_.ins, bt.ins, sync=True,
                               reason="barrier wait")
                post_waits.append((bw_, bar_sem, bar_count[0]))
                last_asm = {}          # j -> last assembly inst
                hr = hr_tiles[0]
                # j = 0: local copy of own shard
                last_asm[0] = nc.sync.dma_start(ht4[:, 0, :, :], hr[:, :, :])
                # j = 1..7: one RDMA broadcast per phase
                prev_g = bw_
                for j in range(1, n_cores):
                    rd = [None] * n_cores
                    rd[j] = (0, j)
                    rx = rxo.tile([128, nt, D], bf16, tag=f"rx{j % 2}")
                    bi = nc.gpsimd.remote_dma_broadcast(
                        rx[:, :, :], hr[:, :, :],
                        rx_sems[j], tx_sem, rdests=rd, queue_num=RQ)
                    add_dep_helper(bi.ins, prev_g.ins, sync=True,
                                   reason="bcast after barrier")
                    tg = nc.gpsimd.trigger_dma(count=None, queue_num=RQ)
                    last_tg = tg
                    rxj_count[j] += 2
                    # placeholder arrival gate; threshold patched
                    # post-scheduling (see post_waits)
                    wi = nc.sync.wait_ge(rx_sems[j], 0)
                    add_dep_helper(wi.ins, tg.ins, sync=True,
                                   reason="gate after trigger")
                    post_waits.append((wi, rx_sems[j], rxj_count[j]))
                    ai = nc.sync.dma_start(ht4[:, j, :, :], rx[:, :, :])
                    add_dep_helper(ai.ins, wi.ins, sync=True,
                                   reason="assemble after gate")
                    last_asm[j] = ai
                rl2 = nc.gpsimd.load_library(mlp)
                add_dep_helper(rl2.ins, last_tg.ins, sync=True,
                               reason="lib back after last trigger")
                # queue q gathers gate on blocks {2q, 2q+1}
                deps = {q: [last_asm[min(2 * q + 1, n_cores - 1)], rl2]
                        for q in range(NQ)}
                return deps

            def layer(ss, idx_sb, dstv_sb, invv_sb, src_tabs, selfT_src,
                      wl, wr, bias, is_last, ag_insts, hr_tiles=None,
                      hT_tiles=None, self_sb=None):
                """Emit one SAGE layer. Returns list of h-publish instrs."""
                gpad = ss["gpad"]; goff = ss["goff"]
                call_list = ss["call_list"]; max_slots = ss["max_slots"]
                store_insts = []
                gather_insts = []
                for si_idx, tiles in enumerate(stiles):
                    w = len(tiles)
                    t0 = tiles[0]
                    # issue the stile's 4 gather calls (parallel queues)
                    gbufs = {}
                    for (q, csi, off, n) in call_list:
                        if csi != si_idx:
                            continue
                        g = gpo.tile([128, max_slots, D], bf16, tag="g")
                        slots = n // 128
                        gi = nc.gpsimd.dma_gather(
                            g[:, :slots, :], src_tabs[q][:],
                            idx_sb[:, off // 16:(off + n) // 16],
                            n, n, D, queue_num=q,
                            single_packet=False)
                        gather_insts.append(gi)
                        if first_gather[0]:
                            add_dep_helper(gi.ins, lib_inst.ins, sync=True,
                                           reason="lib before gather")
                            first_gather[0] = False
                        if ag_insts is not None and si_idx == 0:
                            for ag in ag_insts[q]:
                                add_dep_helper(gi.ins, ag.ins, sync=True,
                                               reason="gather after AG")
                        gbufs[q] = (g, off)
                    # whole-stile self features: SBUF-resident (layer 2)
                    # or one DMA (layer 1)
                    if self_sb is not None:
                        selfT = self_sb[si_idx]
                    else:
                        selfT = sfo.tile([128, max_w * 128], bf16, tag="selfT")
                        nc.sync.dma_start(
                            selfT[:, :w * 128],
                            selfT_src[:, t0 * 128:(t0 + w) * 128])
                    # aggregation per tile -> aggT_big slices
                    aggT = apo.tile([128, max_w * 128], bf16, tag="aggT")
                    for ti, t in enumerate(tiles):
                        chunk_cols = []
                        for q in range(4):
                            npads = int(gpad[t, q])
                            if npads == 0:
                                continue
                            g, off = gbufs[q]
                            base_slot = (int(goff[t, q]) - off) // 128
                            for ci in range(npads // 128):
                                col = int(goff[t, q]) // 128 + ci
                                chunk_cols.append((g, base_slot + ci, col))
                        ps = psa.tile([128, 128], f32, tag="psagg")
                        nchunks = len(chunk_cols)
                        for j, (g, slot, col) in enumerate(chunk_cols):
                            m = mpo.tile([128, 128], bf16, tag="m")
                            nc.vector.tensor_scalar(
                                m[:], iota_sb[:],
                                dstv_sb[:, col:col + 1],
                                invv_sb[:, col:col + 1],
                                mybir.AluOpType.is_equal,
                                mybir.AluOpType.mult)
                            nc.tensor.matmul(
                                ps[:], g[:, slot, :], m[:],
                                start=(j == 0), stop=(j == nchunks - 1))
                        nc.scalar.copy(aggT[:, ti * 128:(ti + 1) * 128], ps[:])
                    # dense + epilogue in 512-wide groups
                    hT = None
                    if is_last:
                        o_big = hpo.tile([128, max_w * 128], bf16, tag="o",
                                         name="o_big")
                    else:
                        hT = hpo.tile([128, max_w * 128], bf16, tag="hT")
                        if hT_tiles is not None:
                            hT_tiles.append(hT)
                    for c0 in range(0, w * 128, 512):
                        c1 = min(c0 + 512, w * 128)
                        ph = psh.tile([128, 512], f32, tag="psh")
                        nc.tensor.matmul(ph[:, :c1 - c0], wl[:],
                                         aggT[:, c0:c1],
                                         start=True, stop=False)
                        nc.tensor.matmul(ph[:, :c1 - c0], wr[:],
                                         selfT[:, c0:c1],
                                         start=False, stop=True)
                        if is_last:
                            nc.scalar.activation(o_big[:, c0:c1],
                                                 ph[:, :c1 - c0],
                                                 copyf, bias=bias[:])
                        else:
                            nc.scalar.activation(hT[:, c0:c1], ph[:, :c1 - c0],
                                                 silu, bias=bias[:])
                    if is_last:
                        nc.sync.dma_start(
                            out_t[:, t0 * 128:(t0 + w) * 128],
                            o_big[:, :w * 128])
                    else:
                        if RDMA_AG:
                            hr = hr_tiles[0]
                            hoff = t0
                        else:
                            hr = hro.tile([128, max_w, D], bf16, tag="hrs",
                                          name="hrs")
                            hoff = 0
                        for ti in range(w):
                            pt = pst.tile([128, 128], bf16, tag="pst")
                            nc.tensor.transpose(
                                pt[:], hT[:, ti * 128:(ti + 1) * 128],
                                ident_sb[:])
                            nc.scalar.copy(hr[:, hoff + ti, :], pt[:])
                        if not RDMA_AG:
                            sins = nc.sync.dma_start(
                                h_row[t0 * 128:(t0 + w) * 128, :].rearrange(
                                    "(a p) f -> p a f", p=128),
                                hr[:, :w, :])
                            store_insts.append(sins)
                return store_insts, gather_insts

            for _ in range(iters):
                hr_tiles = ([hro.tile([128, nt, D], bf16, tag="hrbig",
                                      name="hrbig")]
                            if RDMA_AG else [])
                hT_tiles = []
                l1_stores, l1_gathers = layer(
                    s1, idx1_sb, dstv1_sb, invv1_sb, x_tabs, x_selfT,
                    w_sb["w1lt"], w_sb["w1rt"], b_sb["b1"], False, None,
                    hr_tiles=hr_tiles, hT_tiles=hT_tiles)
                if RDMA_AG:
                    ag_deps = rdma_allgather(l1_gathers, hr_tiles)
                else:
                    ag = nc.gpsimd.collective_compute(
                        "AllGather", mybir.AluOpType.bypass,
                        replica_groups=[list(range(n_cores))],
                        ins=[h_row[:, :]],
                        outs=[h_tab[:]])
                    for sins in l1_stores:
                        add_dep_helper(ag.ins, sins.ins, sync=True,
                                       reason="AG after h stores")
                    ag_deps = {q: [ag] for q in range(NQ)}
                layer(s2, idx2_sb, dstv2_sb, invv2_sb, h_tabs, None,
                      w_sb["w2lt"], w_sb["w2rt"], b_sb["b2"], True, ag_deps,
                      self_sb=hT_tiles)

    if RDMA_AG and not bool(int(os.environ.get("K_RX_NOWAIT", "0"))):
        for wi, sem, v in post_waits:
            done = False
            for wt in wi.ins.sync_info.on_wait:
                if wt.id == sem.num:
                    wt.wait_value = v
                    done = True
            assert done, "gate sem wait not found on gate instruction"
    nc.compile()
    return nc


# ----------------------------------------------------------------- host entry
def _prepare(edge_index):
    return _plan(edge_index, N_NODES, NC)


def _in_maps(plan, x, w1l, w1r, b1, w2l, w2r, b2):
    import ml_dtypes
    bf16 = ml_dtypes.bfloat16

    x = np.asarray(x, dtype=np.float32)
    xb = x.astype(bf16)
    nloc = plan["nloc"]
    n_cores = plan["n_cores"]
    n_nodes = plan["n_nodes"]
    slot_of = plan["slot_of"]
    gslot = (np.arange(n_nodes) // nloc) * NSLOT + slot_of

    # slot-ordered global table (shared by all cores)
    tab_full = np.zeros((NSLOT * n_cores, D), dtype=bf16)
    tab_full[gslot] = xb
    tabs = [np.ascontiguousarray(tab_full[q * QTBL:(q + 1) * QTBL])
            for q in range(4)]
    iota = np.broadcast_to(np.arange(128, dtype=np.float32),
                           (128, 128)).astype(bf16)
    ident = np.eye(128, dtype=np.float32).astype(bf16)
    wts = {
        "w1lt": np.ascontiguousarray(np.asarray(w1l, np.float32).T.astype(bf16)),
        "w1rt": np.ascontiguousarray(np.asarray(w1r, np.float32).T.astype(bf16)),
        "w2lt": np.ascontiguousarray(np.asarray(w2l, np.float32).T.astype(bf16)),
        "w2rt": np.ascontiguousarray(np.asarray(w2r, np.float32).T.astype(bf16)),
    }
    maps = []
    for k in range(n_cores):
        xsT = np.zeros((128, NSLOT), dtype=bf16)
        xsT[:, slot_of[k * nloc:(k + 1) * nloc]] = xb[k * nloc:(k + 1) * nloc].T
        m = {
            "x_selfT": xsT,
            "idxs1": _wrap16(plan["s1"]["idx_st"][k]),
        }
        if plan["s2"] is not plan["s1"]:
            m["idxs2"] = _wrap16(plan["s2"]["idx_st"][k])
            m["dstv2"] = _colmajor(plan["s2"]["dst_st"][k])
            m["invv2"] = _colmajor(plan["s2"]["inv_st"][k])
        m.update({
            "dstv1": _colmajor(plan["s1"]["dst_st"][k]),
            "invv1": _colmajor(plan["s1"]["inv_st"][k]),
            "iota": iota, "ident": ident,
            "b1": np.asarray(b1, np.float32).reshape(128, 1),
            "b2": np.asarray(b2, np.float32).reshape(128, 1),
        })
        m.update(wts)
        for q in range(4):
            m[f"x_tab{q}"] = tabs[q]
        maps.append(m)
    return maps


def _run(inputs, iters=1):
    """Compile (cached) and run; returns full [N, D] output."""
    from concourse.bass_utils import run_bass_kernel_spmd

    edge_index = np.asarray(inputs["edge_index"])
    key = ("k", iters, edge_index.shape[1])
    if key not in _cache:
        plan = _prepare(edge_index)
        nc = _build(plan, iters=iters)
        _cache[key] = (plan, nc)
    plan, nc = _cache[key]
    maps = _in_maps(plan, inputs["x"], inputs["W1_l"], inputs["W1_r"],
                    inputs["b1"], inputs["W2_l"], inputs["W2_r"], inputs["b2"])
    res = run_bass_kernel_spmd(nc, maps, core_ids=list(range(plan["n_cores"])))
    nloc = plan["nloc"]
    slot_of = plan["slot_of"]
    outs = [np.asarray(res.results[k]["outT"], dtype=np.float32).T[
                slot_of[k * nloc:(k + 1) * nloc]]
            for k in range(plan["n_cores"])]
    return np.concatenate(outs, axis=0)


def kernel(**inputs) -> np.ndarray:
    return _run(inputs, iters=1)
